# revision 8
# baseline (speedup 1.0000x reference)
"""BWGNN (Bernstein-wavelet GNN) Trainium2 kernel, 8-core SPMD.

Sharding: nodes split 8 ways (graph/data parallel); edges partitioned by dst
shard; tiny weights replicated.  Per round of Laplacian message passing the
node-state table (dinv * f) is AllGathered, then per-edge src rows are
fetched with dma_gather (int16 indices -> the global table is addressed in
<=25344-row chunks) and segment-summed by dst via dma_scatter_add.  A
scatter instruction must not contain two edges with the same dst (the SDMA
CCE read-modify-write races on duplicates - measured on HW), so edges are
"layered": within a (src-chunk, dst) group, edge #k goes to layer k; every
scatter span stays inside one layer.  Spans rotate over NBUF DRAM agg
buffers (Tile WAW serializes per buffer; buffers overlap), summed on-chip.

MLP in/out runs feature-major with stationary-weight matmuls; node-major
states are produced by PE transposes; the three Bernstein filters are fused
into the transposes with scaled identity matrices.  Outputs are written
feature-major [64, shard]; the host transposes and concatenates.
"""

import sys
from contextlib import ExitStack

import numpy as np

try:
    import concourse  # noqa: F401
except ImportError:  # pragma: no cover
    sys.path.insert(0, "/opt/trn_rl_repo")

import concourse.bacc as bacc
import concourse.bass as bass
import concourse.mybir as mybir
import concourse.tile as tile
from concourse.bass_utils import run_bass_kernel_spmd
from concourse.library_config import mlp
from concourse.masks import make_identity

P = 128
F32 = mybir.dt.float32
I16 = mybir.dt.int16


class Cfg:
    def __init__(self, n_nodes, n_edges, in_feats, h_feats, n_cores,
                 max_span_tiles=8, nbuf=4, mm_chunk=512):
        assert n_nodes % n_cores == 0
        self.n_nodes, self.n_edges = n_nodes, n_edges
        self.in_feats, self.h = in_feats, h_feats
        self.nc = n_cores
        self.shard = n_nodes // n_cores
        self.sp = ((self.shard + P - 1) // P) * P      # padded shard
        self.t = self.sp // P                          # node tiles
        self.tp = self.sp + P                          # table rows/core
        self.tbl_rows = self.tp * n_cores
        self.n_chunks = max(1, (self.tbl_rows + 25343) // 25344)
        self.chunk_shards = -(-n_cores // self.n_chunks)  # shards per chunk
        self.chunk = self.chunk_shards * self.tp
        assert self.chunk <= 32640, self.chunk
        self.n_chunks = -(-n_cores // self.chunk_shards)
        self.max_span_tiles = max_span_tiles           # per-instr tile cap
        self.nbuf = nbuf
        self.mm_chunk = mm_chunk


# ---------------------------------------------------------------- host prep

def _per_core_layers(cfg, srcrow, dstloc):
    """-> dict (chunk, layer) -> (src_local_i16[], dst_i16[]) for one core."""
    q = srcrow // cfg.chunk
    out = {}
    for c in range(cfg.n_chunks):
        m = q == c
        sc = (srcrow[m] - c * cfg.chunk).astype(np.int64)
        dc = dstloc[m].astype(np.int64)
        o = np.argsort(dc, kind="stable")
        sc, dc = sc[o], dc[o]
        # rank within dst group
        if len(dc):
            uniq, start = np.unique(dc, return_index=True)
            grp = np.zeros(len(dc), np.int64)
            grp[start] = 1
            grp = np.cumsum(grp) - 1
            k = np.arange(len(dc)) - start[grp]
            kmax = int(k.max()) + 1
        else:
            k = np.zeros(0, np.int64)
            kmax = 0
        for kk in range(kmax):
            sel = k == kk
            out[(c, kk)] = (sc[sel].astype(np.int16), dc[sel].astype(np.int16))
    return out


def _wrap16(x):
    """flat int16 stream -> [128, n/16]: storage[p, col] = x[col*16 + p%16]."""
    assert len(x) % 16 == 0
    return np.tile(x.reshape(-1, 16).T, (8, 1)).copy()


def preprocess(cfg, in_feat, src, dst, W1, b1, W2, b2, W3, b3, W4, b4):
    n = cfg.n_nodes
    deg = np.bincount(dst, minlength=n).astype(np.float32)
    dinv = np.clip(deg, 1.0, None) ** -0.5

    shard_of = dst // cfg.shard
    layers = []
    for c in range(cfg.nc):
        m = shard_of == c
        src_c, dst_c = src[m].astype(np.int64), dst[m].astype(np.int64)
        srcrow = (src_c // cfg.shard) * cfg.tp + (src_c % cfg.shard)
        dstloc = dst_c - c * cfg.shard
        layers.append(_per_core_layers(cfg, srcrow, dstloc))

    # canonical (chunk, layer) sizes = max over cores, cut into spans
    keys = sorted(set().union(*[set(l.keys()) for l in layers]))
    plan = []  # list of bundles: (chunk, btiles, [(toff, ntiles, bufid)])
    buf_rr = 0
    cur_chunk, cur_spans, cur_tiles = None, [], 0

    def flush():
        nonlocal cur_spans, cur_tiles
        if cur_spans:
            plan.append((cur_chunk, cur_tiles, cur_spans))
        cur_spans, cur_tiles = [], 0

    span_src = []  # parallel to spans in stream order: (chunk, layer, pos)
    for (c, kk) in keys:
        size = max(len(l.get((c, kk), ((), ()))[0]) for l in layers)
        pos = 0
        while pos < size:
            take = min(cfg.max_span_tiles * P, size - pos)
            nt = -(-take // P)
            if cur_chunk != c or cur_tiles + nt > cfg.max_span_tiles:
                flush()
                cur_chunk = c
            cur_spans.append((cur_tiles, nt, buf_rr % cfg.nbuf))
            span_src.append((c, kk, pos, nt))
            buf_rr += 1
            cur_tiles += nt
            pos += take
    flush()

    total_tiles = sum(b[1] for b in plan)
    # per-core index streams in canonical layout
    in_maps = []
    zero_local = [s * cfg.tp + cfg.sp for s in range(cfg.nc)]

    def chunk_zero(c):
        for z in zero_local:
            if c * cfg.chunk <= z < (c + 1) * cfg.chunk:
                return z - c * cfg.chunk
        raise AssertionError("no zero row in chunk")

    for core in range(cfg.nc):
        gz = np.zeros(total_tiles * P, np.int16)
        sz = np.full(total_tiles * P, cfg.sp, np.int16)
        base = 0
        si = 0
        for (c, btiles, spans) in plan:
            for (toff, nt, _buf) in spans:
                (cc, kk, pos, nt2) = span_src[si]
                si += 1
                assert cc == c and nt2 == nt
                s_arr, d_arr = layers[core].get((c, kk), (np.zeros(0, np.int16),) * 2)
                seg_s = s_arr[pos:pos + nt * P]
                seg_d = d_arr[pos:pos + nt * P]
                o = base + toff * P
                gz[o:o + nt * P] = chunk_zero(c)
                gz[o:o + len(seg_s)] = seg_s
                sz[o:o + len(seg_d)] = seg_d
            base += btiles * P
        lo, hi = core * cfg.shard, (core + 1) * cfg.shard
        xT = np.zeros((cfg.in_feats, cfg.sp), np.float32)
        xT[:, :cfg.shard] = in_feat[lo:hi].T
        full = np.ones(cfg.sp, np.float32)
        full[:cfg.shard] = dinv[lo:hi]
        dpm = np.ascontiguousarray(full.reshape(cfg.t, P).T)
        in_maps.append({
            "xT": xT, "dinv_pm": dpm,
            "gidx": _wrap16(gz), "sidx": _wrap16(sz),
            "W1": np.asarray(W1, np.float32), "W2": np.asarray(W2, np.float32),
            "W3": np.asarray(W3, np.float32), "W4": np.asarray(W4, np.float32),
            "b1": np.asarray(b1, np.float32).reshape(-1, 1),
            "b2": np.asarray(b2, np.float32).reshape(-1, 1),
            "b3": np.asarray(b3, np.float32).reshape(-1, 1),
            "b4": np.asarray(b4, np.float32).reshape(-1, 1),
        })
    return in_maps, plan, total_tiles


# ---------------------------------------------------------------- builder

def build_nc(cfg, plan, total_tiles):
    H = cfg.h
    idx_cols = total_tiles * 8
    nc = bacc.Bacc("TRN2", target_bir_lowering=False, debug=False,
                   num_devices=cfg.nc)
    xT_d = nc.dram_tensor("xT", [cfg.in_feats, cfg.sp], F32, kind="ExternalInput")
    dinv_d = nc.dram_tensor("dinv_pm", [P, cfg.t], F32, kind="ExternalInput")
    gidx_d = nc.dram_tensor("gidx", [P, idx_cols], I16, kind="ExternalInput")
    sidx_d = nc.dram_tensor("sidx", [P, idx_cols], I16, kind="ExternalInput")
    W_d = {w: nc.dram_tensor(w, [cfg.in_feats if w in ("W1", "W4") else H, H],
                             F32, kind="ExternalInput")
           for w in ("W1", "W2", "W3", "W4")}
    b_d = {b: nc.dram_tensor(b, [H, 1], F32, kind="ExternalInput")
           for b in ("b1", "b2", "b3", "b4")}
    outl_d = nc.dram_tensor("out_l", [H, cfg.sp], F32, kind="ExternalOutput")
    outh_d = nc.dram_tensor("out_h", [H, cfg.sp], F32, kind="ExternalOutput")

    relu = mybir.ActivationFunctionType.Relu
    cp = mybir.ActivationFunctionType.Copy

    with tile.TileContext(nc) as tc, ExitStack() as ctx:
        pers = ctx.enter_context(tc.tile_pool(name="pers", bufs=1))
        dram = ctx.enter_context(tc.tile_pool(name="dram", bufs=1, space="DRAM"))
        io = ctx.enter_context(tc.tile_pool(name="io", bufs=2))
        one = ctx.enter_context(tc.tile_pool(name="one", bufs=1))
        idxp = ctx.enter_context(tc.tile_pool(name="idxp", bufs=3))
        gbp = ctx.enter_context(tc.tile_pool(name="gbp", bufs=2))
        psum = ctx.enter_context(tc.tile_pool(name="psum", bufs=2, space="PSUM"))
        psum1 = ctx.enter_context(tc.tile_pool(name="psum1", bufs=2, space="PSUM"))

        nc.gpsimd.load_library(mlp)

        f0 = pers.tile([P, cfg.t, 64], F32, tag="f0")
        f1 = pers.tile([P, cfg.t, 64], F32, tag="f1")
        f2 = pers.tile([P, cfg.t, 64], F32, tag="f2")
        tbl = pers.tile([P, cfg.t + 1, 64], F32, tag="tbl")
        dinv_s = pers.tile([P, cfg.t], F32, tag="dinv")
        Ws = {w: pers.tile([cfg.in_feats if w in ("W1", "W4") else H, H],
                           F32, tag=w, name=w + "_s")
              for w in ("W1", "W2", "W3", "W4")}
        bs = {b: pers.tile([H, 1], F32, tag=b, name=b + "_s")
              for b in ("b1", "b2", "b3", "b4")}
        ident = pers.tile([P, P], F32, tag="ident")
        sid3 = pers.tile([P, P], F32, tag="sid3")
        sid075 = pers.tile([P, P], F32, tag="sid075")
        sidm15 = pers.tile([P, P], F32, tag="sidm15")

        tb_in = dram.tile([cfg.tp, 64], F32)
        tb_full = dram.tile([cfg.tp * cfg.nc, 64], F32)
        aggb = [dram.tile([cfg.sp + P, 64], F32, name=f"aggb{i}")
                for i in range(cfg.nbuf)]

        for w in Ws:
            nc.sync.dma_start(Ws[w][:], W_d[w][:])
        for b in bs:
            nc.sync.dma_start(bs[b][:], b_d[b][:])
        nc.sync.dma_start(dinv_s[:], dinv_d[:])
        make_identity(nc, ident[:])
        nc.vector.tensor_scalar_mul(sid3[:], ident[:], 3.0)
        nc.vector.tensor_scalar_mul(sid075[:], ident[:], 0.75)
        nc.vector.tensor_scalar_mul(sidm15[:], ident[:], -1.5)
        nc.gpsimd.memset(tbl[:, cfg.t, :], 0.0)
        zrows = (cfg.sp + P) * 64
        zch = 1
        while (zrows // (P * zch)) * P * zch != zrows or zrows // (P * zch) > 4:
            zch += 1
        zcols = zrows // (P * (zrows // (P * zch) if False else 1))
        # pick zcols so that zrows = n * P * zcols with small n
        n_z = 4
        while zrows % (P * n_z) != 0:
            n_z += 1
        zcols = zrows // (P * n_z)
        ztile = pers.tile([P, zcols], F32, tag="ztile")
        nc.gpsimd.memset(ztile[:], 0.0)

        # ---- phase 1: MLP -> f0 node-major
        CH = cfg.mm_chunk
        for j0 in range(0, cfg.sp, CH):
            w = min(CH, cfg.sp - j0)
            xc = io.tile([cfg.in_feats, CH], F32, tag="xc")
            nc.sync.dma_start(xc[:, :w], xT_d[:, j0:j0 + w])
            ps1 = psum.tile([H, CH], F32, tag="A")
            nc.tensor.matmul(ps1[:, :w], Ws["W1"][:], xc[:, :w],
                             start=True, stop=True)
            h1c = io.tile([H, CH], F32, tag="h1c")
            nc.scalar.activation(h1c[:, :w], ps1[:, :w], relu, bias=bs["b1"][:])
            ps2 = psum.tile([H, CH], F32, tag="B")
            nc.tensor.matmul(ps2[:, :w], Ws["W2"][:], h1c[:, :w],
                             start=True, stop=True)
            h2c = io.tile([H, CH], F32, tag="h2c")
            nc.scalar.activation(h2c[:, :w], ps2[:, :w], relu, bias=bs["b2"][:])
            for i in range(w // P):
                t = (j0 + i * P) // P
                ps3 = psum1.tile([P, 64], F32, tag="C")
                nc.tensor.transpose(ps3[:], h2c[:, i * P:(i + 1) * P],
                                    ident[:H, :H])
                nc.scalar.activation(f0[:, t, :], ps3[:], cp)

        # ---- message passing rounds
        for rnd, (fprev, fnext) in enumerate([(f0, f1), (f1, f2)]):
            nc.vector.tensor_tensor(
                tbl[:, :cfg.t, :], fprev[:],
                dinv_s[:, :, None].to_broadcast([P, cfg.t, 64]),
                mybir.AluOpType.mult)
            nc.sync.dma_start(
                tb_in[:].rearrange("(t p) f -> p t f", p=P), tbl[:])
            nc.gpsimd.collective_compute(
                "AllGather", mybir.AluOpType.bypass,
                replica_groups=[list(range(cfg.nc))],
                ins=[tb_in[:]], outs=[tb_full[:]])
            for ab in aggb:
                zf = ab[:].rearrange("r f -> (r f)")
                for zi in range(n_z):
                    nc.sync.dma_start(
                        zf[zi * P * zcols:(zi + 1) * P * zcols]
                        .rearrange("(p x) -> p x", p=P),
                        ztile[:])
            goff = 0
            for (c, btiles, spans) in plan:
                gi = idxp.tile([P, cfg.max_span_tiles * 8], I16, tag="gi")
                si_ = idxp.tile([P, cfg.max_span_tiles * 8], I16, tag="si")
                nc.sync.dma_start(gi[:, :btiles * 8],
                                  gidx_d[:, goff * 8:(goff + btiles) * 8])
                nc.sync.dma_start(si_[:, :btiles * 8],
                                  sidx_d[:, goff * 8:(goff + btiles) * 8])
                gb = gbp.tile([P, cfg.max_span_tiles, 64], F32, tag="gb")
                ni = btiles * P
                nc.gpsimd.dma_gather(
                    gb[:, :btiles, :],
                    tb_full[c * cfg.chunk:(c + 1) * cfg.chunk, :],
                    gi[:, :btiles * 8], ni, ni, 64)
                for (toff, nt, bufid) in spans:
                    nc.gpsimd.dma_scatter_add(
                        aggb[bufid][:], gb[:, toff:toff + nt, :],
                        si_[:, toff * 8:(toff + nt) * 8],
                        nt * P, nt * P, 64)
                goff += btiles
            # merge agg buffers into tbl (reused as scratch), update fnext
            acc = tbl
            for i, ab in enumerate(aggb):
                tmp = one.tile([P, cfg.t, 64], F32, tag="aggin")
                nc.sync.dma_start(
                    tmp[:], ab[:cfg.sp, :].rearrange("(t p) f -> p t f", p=P))
                if i == 0:
                    nc.vector.tensor_copy(acc[:, :cfg.t, :], tmp[:])
                else:
                    nc.vector.tensor_tensor(acc[:, :cfg.t, :],
                                            acc[:, :cfg.t, :], tmp[:],
                                            mybir.AluOpType.add)
            nc.vector.tensor_tensor(
                acc[:, :cfg.t, :], acc[:, :cfg.t, :],
                dinv_s[:, :, None].to_broadcast([P, cfg.t, 64]),
                mybir.AluOpType.mult)
            nc.vector.tensor_tensor(fnext[:], fprev[:], acc[:, :cfg.t, :],
                                    mybir.AluOpType.subtract)

        # ---- filters + output MLPs
        nc.vector.tensor_tensor(f0[:], f0[:], f1[:], mybir.AluOpType.subtract)
        for j0 in range(0, cfg.sp, CH):
            w = min(CH, cfg.sp - j0)
            zl = psum.tile([H, CH], F32, tag="A")
            z1 = psum.tile([H, CH], F32, tag="B")
            z2 = psum1.tile([H, CH], F32, tag="C")
            for i in range(w // P):
                t = (j0 + i * P) // P
                cs = slice(i * P, (i + 1) * P)
                nc.tensor.matmul(zl[:, cs], f0[:, t, :], sid3[:],
                                 start=True, stop=False)
                nc.tensor.matmul(zl[:, cs], f2[:, t, :], sid075[:],
                                 start=False, stop=True)
                nc.tensor.matmul(z1[:, cs], f1[:, t, :], sid3[:],
                                 start=True, stop=False)
                nc.tensor.matmul(z1[:, cs], f2[:, t, :], sidm15[:],
                                 start=False, stop=True)
                nc.tensor.matmul(z2[:, cs], f2[:, t, :], sid075[:],
                                 start=True, stop=True)
            zlc = io.tile([H, CH], F32, tag="zlc")
            zhc = io.tile([P, CH], F32, tag="zhc")
            nc.scalar.activation(zlc[:, :w], zl[:, :w], cp)
            nc.scalar.activation(zhc[:H, :w], z1[:, :w], cp)
            nc.scalar.activation(zhc[H:, :w], z2[:, :w], cp)
            pl = psum1.tile([H, CH], F32, tag="C")
            ph = psum.tile([H, CH], F32, tag="A")
            nc.tensor.matmul(pl[:, :w], Ws["W3"][:], zlc[:, :w],
                             start=True, stop=True)
            nc.tensor.matmul(ph[:, :w], Ws["W4"][:], zhc[:, :w],
                             start=True, stop=True)
            ol = io.tile([H, CH], F32, tag="ol")
            oh = io.tile([H, CH], F32, tag="oh")
            nc.scalar.activation(ol[:, :w], pl[:, :w], relu, bias=bs["b3"][:])
            nc.scalar.activation(oh[:, :w], ph[:, :w], relu, bias=bs["b4"][:])
            nc.sync.dma_start(outl_d[:, j0:j0 + w], ol[:, :w])
            nc.sync.dma_start(outh_d[:, j0:j0 + w], oh[:, :w])

    nc.compile()
    return nc


# ---------------------------------------------------------------- driver

_CACHE = {}


def run(cfg, inputs, run_fn=None, **spmd_kwargs):
    in_maps, plan, total_tiles = preprocess(cfg, **inputs)
    key = (cfg.n_nodes, cfg.n_edges, total_tiles,
           tuple((c, b, tuple(s)) for c, b, s in plan))
    if key not in _CACHE:
        _CACHE[key] = build_nc(cfg, plan, total_tiles)
    nc = _CACHE[key]
    if run_fn is not None:
        results = run_fn(nc, in_maps)
        res = None
    else:
        res = run_bass_kernel_spmd(nc, in_maps, core_ids=list(range(cfg.nc)), **spmd_kwargs)
        results = res.results
    h_l = np.zeros((cfg.n_nodes, cfg.h), np.float32)
    h_h = np.zeros((cfg.n_nodes, cfg.h), np.float32)
    for c in range(cfg.nc):
        lo = c * cfg.shard
        h_l[lo:lo + cfg.shard] = results[c]["out_l"].T[:cfg.shard]
        h_h[lo:lo + cfg.shard] = results[c]["out_h"].T[:cfg.shard]
    return h_l, h_h, res


def kernel(in_feat, src, dst, W1, b1, W2, b2, W3, b3, W4, b4):
    cfg = Cfg(100000, 1600000, 128, 64, 8)
    h_l, h_h, _ = run(cfg, dict(
        in_feat=np.asarray(in_feat, np.float32),
        src=np.asarray(src, np.int64), dst=np.asarray(dst, np.int64),
        W1=np.asarray(W1, np.float32), b1=np.asarray(b1, np.float32),
        W2=np.asarray(W2, np.float32), b2=np.asarray(b2, np.float32),
        W3=np.asarray(W3, np.float32), b3=np.asarray(b3, np.float32),
        W4=np.asarray(W4, np.float32), b4=np.asarray(b4, np.float32)))
    return h_l, h_h


# revision 10
# speedup vs baseline: 1.0194x; 1.0194x over previous
"""BWGNN (Bernstein-wavelet GNN) Trainium2 kernel, 8-core SPMD.

Sharding: nodes split 8 ways (graph/data parallel); edges partitioned by dst
shard; tiny weights replicated.  Per round of Laplacian message passing the
node-state table (dinv * f) is AllGathered, then per-edge src rows are
fetched with dma_gather (int16 indices -> the global table is addressed in
<=25344-row chunks) and segment-summed by dst via dma_scatter_add.  A
scatter instruction must not contain two edges with the same dst (the SDMA
CCE read-modify-write races on duplicates - measured on HW), so edges are
"layered": within a (src-chunk, dst) group, edge #k goes to layer k; every
scatter span stays inside one layer.  Spans rotate over NBUF DRAM agg
buffers (Tile WAW serializes per buffer; buffers overlap), summed on-chip.

MLP in/out runs feature-major with stationary-weight matmuls; node-major
states are produced by PE transposes; the three Bernstein filters are fused
into the transposes with scaled identity matrices.  Outputs are written
feature-major [64, shard]; the host transposes and concatenates.
"""

import sys
from contextlib import ExitStack

import numpy as np

try:
    import concourse  # noqa: F401
except ImportError:  # pragma: no cover
    sys.path.insert(0, "/opt/trn_rl_repo")

import concourse.bacc as bacc
import concourse.bass as bass
import concourse.mybir as mybir
import concourse.tile as tile
from concourse.bass_utils import run_bass_kernel_spmd
from concourse.library_config import mlp
from concourse.masks import make_identity

P = 128
F32 = mybir.dt.float32
I16 = mybir.dt.int16


class Cfg:
    def __init__(self, n_nodes, n_edges, in_feats, h_feats, n_cores,
                 max_span_tiles=8, nbuf=4, mm_chunk=512):
        assert n_nodes % n_cores == 0
        self.n_nodes, self.n_edges = n_nodes, n_edges
        self.in_feats, self.h = in_feats, h_feats
        self.nc = n_cores
        self.shard = n_nodes // n_cores
        self.sp = ((self.shard + P - 1) // P) * P      # padded shard
        self.t = self.sp // P                          # node tiles
        self.tp = self.sp + P                          # table rows/core
        self.tbl_rows = self.tp * n_cores
        self.n_chunks = max(1, (self.tbl_rows + 25343) // 25344)
        self.chunk_shards = -(-n_cores // self.n_chunks)  # shards per chunk
        self.chunk = self.chunk_shards * self.tp
        assert self.chunk <= 32640, self.chunk
        self.n_chunks = -(-n_cores // self.chunk_shards)
        self.max_span_tiles = max_span_tiles           # per-instr tile cap
        self.nbuf = nbuf
        self.mm_chunk = mm_chunk


# ---------------------------------------------------------------- host prep

def _per_core_layers(cfg, srcrow, dstloc):
    """-> dict (chunk, layer) -> (src_local_i16[], dst_i16[]) for one core."""
    q = srcrow // cfg.chunk
    out = {}
    for c in range(cfg.n_chunks):
        m = q == c
        sc = (srcrow[m] - c * cfg.chunk).astype(np.int64)
        dc = dstloc[m].astype(np.int64)
        o = np.argsort(dc, kind="stable")
        sc, dc = sc[o], dc[o]
        # rank within dst group
        if len(dc):
            uniq, start = np.unique(dc, return_index=True)
            grp = np.zeros(len(dc), np.int64)
            grp[start] = 1
            grp = np.cumsum(grp) - 1
            k = np.arange(len(dc)) - start[grp]
            kmax = int(k.max()) + 1
        else:
            k = np.zeros(0, np.int64)
            kmax = 0
        for kk in range(kmax):
            sel = k == kk
            out[(c, kk)] = (sc[sel].astype(np.int16), dc[sel].astype(np.int16))
    return out


def _wrap16(x):
    """flat int16 stream -> [128, n/16]: storage[p, col] = x[col*16 + p%16]."""
    assert len(x) % 16 == 0
    return np.tile(x.reshape(-1, 16).T, (8, 1)).copy()


def preprocess(cfg, in_feat, src, dst, W1, b1, W2, b2, W3, b3, W4, b4):
    n = cfg.n_nodes
    deg = np.bincount(dst, minlength=n).astype(np.float32)
    dinv = np.clip(deg, 1.0, None) ** -0.5

    shard_of = dst // cfg.shard
    layers = []
    for c in range(cfg.nc):
        m = shard_of == c
        src_c, dst_c = src[m].astype(np.int64), dst[m].astype(np.int64)
        srcrow = (src_c // cfg.shard) * cfg.tp + (src_c % cfg.shard)
        dstloc = dst_c - c * cfg.shard
        layers.append(_per_core_layers(cfg, srcrow, dstloc))

    # canonical (chunk, layer) sizes = max over cores, cut into spans
    keys = sorted(set().union(*[set(l.keys()) for l in layers]))
    plan = []  # list of bundles: (chunk, btiles, [(toff, ntiles, bufid)])
    buf_rr = 0
    cur_chunk, cur_spans, cur_tiles = None, [], 0

    def flush():
        nonlocal cur_spans, cur_tiles
        if cur_spans:
            plan.append((cur_chunk, cur_tiles, cur_spans))
        cur_spans, cur_tiles = [], 0

    span_src = []  # parallel to spans in stream order: (chunk, layer, pos)
    for (c, kk) in keys:
        size = max(len(l.get((c, kk), ((), ()))[0]) for l in layers)
        pos = 0
        while pos < size:
            take = min(cfg.max_span_tiles * P, size - pos)
            nt = -(-take // P)
            if cur_chunk != c or cur_tiles + nt > cfg.max_span_tiles:
                flush()
                cur_chunk = c
            cur_spans.append((cur_tiles, nt, buf_rr % cfg.nbuf))
            span_src.append((c, kk, pos, nt))
            buf_rr += 1
            cur_tiles += nt
            pos += take
    flush()

    total_tiles = sum(b[1] for b in plan)
    # per-core index streams in canonical layout
    in_maps = []
    zero_local = [s * cfg.tp + cfg.sp for s in range(cfg.nc)]

    def chunk_zero(c):
        for z in zero_local:
            if c * cfg.chunk <= z < (c + 1) * cfg.chunk:
                return z - c * cfg.chunk
        raise AssertionError("no zero row in chunk")

    for core in range(cfg.nc):
        gz = np.zeros(total_tiles * P, np.int16)
        sz = np.full(total_tiles * P, cfg.sp, np.int16)
        base = 0
        si = 0
        for (c, btiles, spans) in plan:
            for (toff, nt, _buf) in spans:
                (cc, kk, pos, nt2) = span_src[si]
                si += 1
                assert cc == c and nt2 == nt
                s_arr, d_arr = layers[core].get((c, kk), (np.zeros(0, np.int16),) * 2)
                seg_s = s_arr[pos:pos + nt * P]
                seg_d = d_arr[pos:pos + nt * P]
                o = base + toff * P
                gz[o:o + nt * P] = chunk_zero(c)
                gz[o:o + len(seg_s)] = seg_s
                sz[o:o + len(seg_d)] = seg_d
            base += btiles * P
        lo, hi = core * cfg.shard, (core + 1) * cfg.shard
        xT = np.zeros((cfg.in_feats, cfg.sp), np.float32)
        xT[:, :cfg.shard] = in_feat[lo:hi].T
        full = np.ones(cfg.sp, np.float32)
        full[:cfg.shard] = dinv[lo:hi]
        dpm = np.ascontiguousarray(full.reshape(cfg.t, P).T)
        in_maps.append({
            "xT": xT, "dinv_pm": dpm,
            "gidx": _wrap16(gz), "sidx": _wrap16(sz),
            "W1": np.asarray(W1, np.float32), "W2": np.asarray(W2, np.float32),
            "W3": np.asarray(W3, np.float32), "W4": np.asarray(W4, np.float32),
            "b1": np.asarray(b1, np.float32).reshape(-1, 1),
            "b2": np.asarray(b2, np.float32).reshape(-1, 1),
            "b3": np.asarray(b3, np.float32).reshape(-1, 1),
            "b4": np.asarray(b4, np.float32).reshape(-1, 1),
        })
    return in_maps, plan, total_tiles


# ---------------------------------------------------------------- builder

def build_nc(cfg, plan, total_tiles):
    H = cfg.h
    idx_cols = total_tiles * 8
    nc = bacc.Bacc("TRN2", target_bir_lowering=False, debug=False,
                   num_devices=cfg.nc)
    xT_d = nc.dram_tensor("xT", [cfg.in_feats, cfg.sp], F32, kind="ExternalInput")
    dinv_d = nc.dram_tensor("dinv_pm", [P, cfg.t], F32, kind="ExternalInput")
    gidx_d = nc.dram_tensor("gidx", [P, idx_cols], I16, kind="ExternalInput")
    sidx_d = nc.dram_tensor("sidx", [P, idx_cols], I16, kind="ExternalInput")
    W_d = {w: nc.dram_tensor(w, [cfg.in_feats if w in ("W1", "W4") else H, H],
                             F32, kind="ExternalInput")
           for w in ("W1", "W2", "W3", "W4")}
    b_d = {b: nc.dram_tensor(b, [H, 1], F32, kind="ExternalInput")
           for b in ("b1", "b2", "b3", "b4")}
    outl_d = nc.dram_tensor("out_l", [H, cfg.sp], F32, kind="ExternalOutput")
    outh_d = nc.dram_tensor("out_h", [H, cfg.sp], F32, kind="ExternalOutput")

    relu = mybir.ActivationFunctionType.Relu
    cp = mybir.ActivationFunctionType.Copy

    with tile.TileContext(nc) as tc, ExitStack() as ctx:
        pers = ctx.enter_context(tc.tile_pool(name="pers", bufs=1))
        dram = ctx.enter_context(tc.tile_pool(name="dram", bufs=1, space="DRAM"))
        io = ctx.enter_context(tc.tile_pool(name="io", bufs=2))
        one = ctx.enter_context(tc.tile_pool(name="one", bufs=1))
        idxp = ctx.enter_context(tc.tile_pool(name="idxp", bufs=6))
        gbp = ctx.enter_context(tc.tile_pool(name="gbp", bufs=6))
        psum = ctx.enter_context(tc.tile_pool(name="psum", bufs=2, space="PSUM"))
        psum1 = ctx.enter_context(tc.tile_pool(name="psum1", bufs=2, space="PSUM"))

        nc.gpsimd.load_library(mlp)

        f0 = pers.tile([P, cfg.t, 64], F32, tag="f0")
        f1 = pers.tile([P, cfg.t, 64], F32, tag="f1")
        f2 = pers.tile([P, cfg.t, 64], F32, tag="f2")
        tbl = pers.tile([P, cfg.t + 1, 64], F32, tag="tbl")
        dinv_s = pers.tile([P, cfg.t], F32, tag="dinv")
        Ws = {w: pers.tile([cfg.in_feats if w in ("W1", "W4") else H, H],
                           F32, tag=w, name=w + "_s")
              for w in ("W1", "W2", "W3", "W4")}
        bs = {b: pers.tile([H, 1], F32, tag=b, name=b + "_s")
              for b in ("b1", "b2", "b3", "b4")}
        ident = pers.tile([P, P], F32, tag="ident")
        sid3 = pers.tile([P, P], F32, tag="sid3")
        sid075 = pers.tile([P, P], F32, tag="sid075")
        sidm15 = pers.tile([P, P], F32, tag="sidm15")

        tb_ins = [dram.tile([cfg.tp, 64], F32, name=f"tb_in{r}")
                  for r in range(2)]
        tb_fulls = [dram.tile([cfg.tp * cfg.nc, 64], F32, addr_space="Shared",
                              name=f"tb_full{r}") for r in range(2)]
        aggb = [dram.tile([cfg.sp + P, 64], F32, name=f"aggb{i}")
                for i in range(cfg.nbuf)]

        for w in Ws:
            nc.sync.dma_start(Ws[w][:], W_d[w][:])
        for b in bs:
            nc.sync.dma_start(bs[b][:], b_d[b][:])
        nc.sync.dma_start(dinv_s[:], dinv_d[:])
        make_identity(nc, ident[:])
        nc.vector.tensor_scalar_mul(sid3[:], ident[:], 3.0)
        nc.vector.tensor_scalar_mul(sid075[:], ident[:], 0.75)
        nc.vector.tensor_scalar_mul(sidm15[:], ident[:], -1.5)
        nc.gpsimd.memset(tbl[:, cfg.t, :], 0.0)
        zrows = (cfg.sp + P) * 64
        zch = 1
        while (zrows // (P * zch)) * P * zch != zrows or zrows // (P * zch) > 4:
            zch += 1
        zcols = zrows // (P * (zrows // (P * zch) if False else 1))
        # pick zcols so that zrows = n * P * zcols with small n
        n_z = 4
        while zrows % (P * n_z) != 0:
            n_z += 1
        zcols = zrows // (P * n_z)
        ztile = pers.tile([P, zcols], F32, tag="ztile")
        nc.gpsimd.memset(ztile[:], 0.0)

        # ---- phase 1: MLP -> f0 node-major
        CH = cfg.mm_chunk
        for j0 in range(0, cfg.sp, CH):
            w = min(CH, cfg.sp - j0)
            xc = io.tile([cfg.in_feats, CH], F32, tag="xc")
            nc.sync.dma_start(xc[:, :w], xT_d[:, j0:j0 + w])
            ps1 = psum.tile([H, CH], F32, tag="A")
            nc.tensor.matmul(ps1[:, :w], Ws["W1"][:], xc[:, :w],
                             start=True, stop=True)
            h1c = io.tile([H, CH], F32, tag="h1c")
            nc.scalar.activation(h1c[:, :w], ps1[:, :w], relu, bias=bs["b1"][:])
            ps2 = psum.tile([H, CH], F32, tag="B")
            nc.tensor.matmul(ps2[:, :w], Ws["W2"][:], h1c[:, :w],
                             start=True, stop=True)
            h2c = io.tile([H, CH], F32, tag="h2c")
            nc.scalar.activation(h2c[:, :w], ps2[:, :w], relu, bias=bs["b2"][:])
            for i in range(w // P):
                t = (j0 + i * P) // P
                ps3 = psum1.tile([P, 64], F32, tag="C")
                nc.tensor.transpose(ps3[:], h2c[:, i * P:(i + 1) * P],
                                    ident[:H, :H])
                nc.scalar.activation(f0[:, t, :], ps3[:], cp)

        # ---- message passing rounds
        for rnd, (fprev, fnext) in enumerate([(f0, f1), (f1, f2)]):
            tb_in, tb_full = tb_ins[rnd], tb_fulls[rnd]
            nc.vector.tensor_tensor(
                tbl[:, :cfg.t, :], fprev[:],
                dinv_s[:, :, None].to_broadcast([P, cfg.t, 64]),
                mybir.AluOpType.mult)
            nc.sync.dma_start(
                tb_in[:].rearrange("(t p) f -> p t f", p=P), tbl[:])
            nc.gpsimd.collective_compute(
                "AllGather", mybir.AluOpType.bypass,
                replica_groups=[list(range(cfg.nc))],
                ins=[tb_in[:]], outs=[tb_full[:]])
            for ab in aggb:
                zf = ab[:].rearrange("r f -> (r f)")
                for zi in range(n_z):
                    nc.sync.dma_start(
                        zf[zi * P * zcols:(zi + 1) * P * zcols]
                        .rearrange("(p x) -> p x", p=P),
                        ztile[:])
            goff = 0
            for (c, btiles, spans) in plan:
                gi = idxp.tile([P, cfg.max_span_tiles * 8], I16, tag="gi")
                si_ = idxp.tile([P, cfg.max_span_tiles * 8], I16, tag="si")
                nc.sync.dma_start(gi[:, :btiles * 8],
                                  gidx_d[:, goff * 8:(goff + btiles) * 8])
                nc.sync.dma_start(si_[:, :btiles * 8],
                                  sidx_d[:, goff * 8:(goff + btiles) * 8])
                gb = gbp.tile([P, cfg.max_span_tiles, 64], F32, tag="gb")
                ni = btiles * P
                nc.gpsimd.dma_gather(
                    gb[:, :btiles, :],
                    tb_full[c * cfg.chunk:(c + 1) * cfg.chunk, :],
                    gi[:, :btiles * 8], ni, ni, 64)
                for (toff, nt, bufid) in spans:
                    nc.gpsimd.dma_scatter_add(
                        aggb[bufid][:], gb[:, toff:toff + nt, :],
                        si_[:, toff * 8:(toff + nt) * 8],
                        nt * P, nt * P, 64)
                goff += btiles
            # merge agg buffers into tbl (reused as scratch), update fnext
            acc = tbl
            for i, ab in enumerate(aggb):
                tmp = one.tile([P, cfg.t, 64], F32, tag="aggin")
                nc.sync.dma_start(
                    tmp[:], ab[:cfg.sp, :].rearrange("(t p) f -> p t f", p=P))
                if i == 0:
                    nc.vector.tensor_copy(acc[:, :cfg.t, :], tmp[:])
                else:
                    nc.vector.tensor_tensor(acc[:, :cfg.t, :],
                                            acc[:, :cfg.t, :], tmp[:],
                                            mybir.AluOpType.add)
            nc.vector.tensor_tensor(
                acc[:, :cfg.t, :], acc[:, :cfg.t, :],
                dinv_s[:, :, None].to_broadcast([P, cfg.t, 64]),
                mybir.AluOpType.mult)
            nc.vector.tensor_tensor(fnext[:], fprev[:], acc[:, :cfg.t, :],
                                    mybir.AluOpType.subtract)

        # ---- filters + output MLPs
        nc.vector.tensor_tensor(f0[:], f0[:], f1[:], mybir.AluOpType.subtract)
        for j0 in range(0, cfg.sp, CH):
            w = min(CH, cfg.sp - j0)
            zl = psum.tile([H, CH], F32, tag="A")
            z1 = psum.tile([H, CH], F32, tag="B")
            z2 = psum1.tile([H, CH], F32, tag="C")
            for i in range(w // P):
                t = (j0 + i * P) // P
                cs = slice(i * P, (i + 1) * P)
                nc.tensor.matmul(zl[:, cs], f0[:, t, :], sid3[:],
                                 start=True, stop=False)
                nc.tensor.matmul(zl[:, cs], f2[:, t, :], sid075[:],
                                 start=False, stop=True)
                nc.tensor.matmul(z1[:, cs], f1[:, t, :], sid3[:],
                                 start=True, stop=False)
                nc.tensor.matmul(z1[:, cs], f2[:, t, :], sidm15[:],
                                 start=False, stop=True)
                nc.tensor.matmul(z2[:, cs], f2[:, t, :], sid075[:],
                                 start=True, stop=True)
            zlc = io.tile([H, CH], F32, tag="zlc")
            zhc = io.tile([P, CH], F32, tag="zhc")
            nc.scalar.activation(zlc[:, :w], zl[:, :w], cp)
            nc.scalar.activation(zhc[:H, :w], z1[:, :w], cp)
            nc.scalar.activation(zhc[H:, :w], z2[:, :w], cp)
            pl = psum1.tile([H, CH], F32, tag="C")
            ph = psum.tile([H, CH], F32, tag="A")
            nc.tensor.matmul(pl[:, :w], Ws["W3"][:], zlc[:, :w],
                             start=True, stop=True)
            nc.tensor.matmul(ph[:, :w], Ws["W4"][:], zhc[:, :w],
                             start=True, stop=True)
            ol = io.tile([H, CH], F32, tag="ol")
            oh = io.tile([H, CH], F32, tag="oh")
            nc.scalar.activation(ol[:, :w], pl[:, :w], relu, bias=bs["b3"][:])
            nc.scalar.activation(oh[:, :w], ph[:, :w], relu, bias=bs["b4"][:])
            nc.sync.dma_start(outl_d[:, j0:j0 + w], ol[:, :w])
            nc.sync.dma_start(outh_d[:, j0:j0 + w], oh[:, :w])

    nc.compile()
    return nc


# ---------------------------------------------------------------- driver

_CACHE = {}


def run(cfg, inputs, run_fn=None, **spmd_kwargs):
    in_maps, plan, total_tiles = preprocess(cfg, **inputs)
    key = (cfg.n_nodes, cfg.n_edges, total_tiles,
           tuple((c, b, tuple(s)) for c, b, s in plan))
    if key not in _CACHE:
        _CACHE[key] = build_nc(cfg, plan, total_tiles)
    nc = _CACHE[key]
    if run_fn is not None:
        results = run_fn(nc, in_maps)
        res = None
    else:
        res = run_bass_kernel_spmd(nc, in_maps, core_ids=list(range(cfg.nc)), **spmd_kwargs)
        results = res.results
    h_l = np.zeros((cfg.n_nodes, cfg.h), np.float32)
    h_h = np.zeros((cfg.n_nodes, cfg.h), np.float32)
    for c in range(cfg.nc):
        lo = c * cfg.shard
        h_l[lo:lo + cfg.shard] = results[c]["out_l"].T[:cfg.shard]
        h_h[lo:lo + cfg.shard] = results[c]["out_h"].T[:cfg.shard]
    return h_l, h_h, res


def kernel(in_feat, src, dst, W1, b1, W2, b2, W3, b3, W4, b4):
    cfg = Cfg(100000, 1600000, 128, 64, 8)
    h_l, h_h, _ = run(cfg, dict(
        in_feat=np.asarray(in_feat, np.float32),
        src=np.asarray(src, np.int64), dst=np.asarray(dst, np.int64),
        W1=np.asarray(W1, np.float32), b1=np.asarray(b1, np.float32),
        W2=np.asarray(W2, np.float32), b2=np.asarray(b2, np.float32),
        W3=np.asarray(W3, np.float32), b3=np.asarray(b3, np.float32),
        W4=np.asarray(W4, np.float32), b4=np.asarray(b4, np.float32)))
    return h_l, h_h


# revision 14
# speedup vs baseline: 1.7690x; 1.7353x over previous
"""BWGNN (Bernstein-wavelet GNN) Trainium2 kernel, 8-core SPMD.

Sharding: nodes split 8 ways (graph/data parallel); edges partitioned by dst
shard; tiny weights replicated.  Per round of Laplacian message passing the
node-state table (dinv * f) is AllGathered, then per-edge src rows are
fetched with dma_gather (int16 indices -> the global table is addressed in
<=25344-row chunks) and segment-summed by dst via dma_scatter_add.  A
scatter instruction must not contain two edges with the same dst (the SDMA
CCE read-modify-write races on duplicates - measured on HW), so edges are
"layered": within a (src-chunk, dst) group, edge #k goes to layer k; every
scatter span stays inside one layer.  Spans rotate over NBUF DRAM agg
buffers (Tile WAW serializes per buffer; buffers overlap), summed on-chip.

MLP in/out runs feature-major with stationary-weight matmuls; node-major
states are produced by PE transposes; the three Bernstein filters are fused
into the transposes with scaled identity matrices.  Outputs are written
feature-major [64, shard]; the host transposes and concatenates.
"""

import sys
from contextlib import ExitStack

import numpy as np

try:
    import concourse  # noqa: F401
except ImportError:  # pragma: no cover
    sys.path.insert(0, "/opt/trn_rl_repo")

import concourse.bacc as bacc
import concourse.bass as bass
import concourse.mybir as mybir
import concourse.tile as tile
from concourse.bass_utils import run_bass_kernel_spmd
from concourse.library_config import mlp
from concourse.masks import make_identity

P = 128
F32 = mybir.dt.float32
I16 = mybir.dt.int16


class Cfg:
    def __init__(self, n_nodes, n_edges, in_feats, h_feats, n_cores,
                 max_span_tiles=8, nbuf=4, mm_chunk=512):
        assert n_nodes % n_cores == 0
        self.n_nodes, self.n_edges = n_nodes, n_edges
        self.in_feats, self.h = in_feats, h_feats
        self.nc = n_cores
        self.shard = n_nodes // n_cores
        self.sp = ((self.shard + P - 1) // P) * P      # padded shard
        self.t = self.sp // P                          # node tiles
        self.tp = self.sp + P                          # table rows/core
        self.tbl_rows = self.tp * n_cores
        self.n_chunks = max(1, (self.tbl_rows + 25343) // 25344)
        self.chunk_shards = -(-n_cores // self.n_chunks)  # shards per chunk
        self.chunk = self.chunk_shards * self.tp
        assert self.chunk <= 32640, self.chunk
        self.n_chunks = -(-n_cores // self.chunk_shards)
        self.max_span_tiles = max_span_tiles           # per-instr tile cap
        self.nbuf = nbuf
        self.mm_chunk = mm_chunk


# ---------------------------------------------------------------- host prep

def _per_core_groups(cfg, srcrow, dstloc):
    """-> dict (chunk, window) -> (src_local_i16[], dst_in_window_f32[])."""
    q = srcrow // cfg.chunk
    out = {}
    for c in range(cfg.n_chunks):
        m = q == c
        sc = (srcrow[m] - c * cfg.chunk).astype(np.int64)
        dc = dstloc[m].astype(np.int64)
        o = np.argsort(dc, kind="stable")
        sc, dc = sc[o], dc[o]
        w = dc // P
        for ww in np.unique(w):
            sel = w == ww
            out[(c, int(ww))] = (sc[sel].astype(np.int16),
                                 (dc[sel] % P).astype(np.float32))
    return out


def _wrap16(x):
    """flat int16 stream -> [128, n/16]: storage[p, col] = x[col*16 + p%16]."""
    assert len(x) % 16 == 0
    return np.tile(x.reshape(-1, 16).T, (8, 1)).copy()


def preprocess(cfg, in_feat, src, dst, W1, b1, W2, b2, W3, b3, W4, b4):
    n = cfg.n_nodes
    deg = np.bincount(dst, minlength=n).astype(np.float32)
    dinv = np.clip(deg, 1.0, None) ** -0.5

    shard_of = dst // cfg.shard
    groups = []
    for c in range(cfg.nc):
        m = shard_of == c
        src_c, dst_c = src[m].astype(np.int64), dst[m].astype(np.int64)
        srcrow = (src_c // cfg.shard) * cfg.tp + (src_c % cfg.shard)
        dstloc = dst_c - c * cfg.shard
        groups.append(_per_core_groups(cfg, srcrow, dstloc))

    # canonical (chunk, window) sizes = max over cores, cut into spans
    keys = sorted(set().union(*[set(g.keys()) for g in groups]))
    plan = []  # bundles: (chunk, btiles, [(toff, ntiles, window)])
    cur_chunk, cur_spans, cur_tiles = None, [], 0

    def flush():
        nonlocal cur_spans, cur_tiles
        if cur_spans:
            plan.append((cur_chunk, cur_tiles, cur_spans))
        cur_spans, cur_tiles = [], 0

    span_src = []  # (chunk, window, pos, ntiles) in stream order
    for (c, ww) in keys:
        size = max(len(g.get((c, ww), ((), ()))[0]) for g in groups)
        pos = 0
        while pos < size:
            take = min(cfg.max_span_tiles * P, size - pos)
            nt = -(-take // P)
            if cur_chunk != c or cur_tiles + nt > cfg.max_span_tiles:
                flush()
                cur_chunk = c
            cur_spans.append((cur_tiles, nt, ww))
            span_src.append((c, ww, pos, nt))
            cur_tiles += nt
            pos += take
    flush()

    total_tiles = sum(b[1] for b in plan)
    in_maps = []
    zero_local = [s * cfg.tp + cfg.sp for s in range(cfg.nc)]

    def chunk_zero(c):
        for z in zero_local:
            if c * cfg.chunk <= z < (c + 1) * cfg.chunk:
                return z - c * cfg.chunk
        raise AssertionError("no zero row in chunk")

    for core in range(cfg.nc):
        gz = np.zeros(total_tiles * P, np.int16)
        dw = np.full(total_tiles * P, 999.0, np.float32)
        base = 0
        si = 0
        for (c, btiles, spans) in plan:
            for (toff, nt, ww) in spans:
                (cc, ww2, pos, nt2) = span_src[si]
                si += 1
                assert cc == c and nt2 == nt and ww2 == ww
                s_arr, d_arr = groups[core].get(
                    (c, ww), (np.zeros(0, np.int16), np.zeros(0, np.float32)))
                seg_s = s_arr[pos:pos + nt * P]
                seg_d = d_arr[pos:pos + nt * P]
                o = base + toff * P
                gz[o:o + nt * P] = chunk_zero(c)
                gz[o:o + len(seg_s)] = seg_s
                dw[o:o + len(seg_d)] = seg_d
            base += btiles * P
        lo, hi = core * cfg.shard, (core + 1) * cfg.shard
        xT = np.zeros((cfg.in_feats, cfg.sp), np.float32)
        xT[:, :cfg.shard] = in_feat[lo:hi].T
        full = np.ones(cfg.sp, np.float32)
        full[:cfg.shard] = dinv[lo:hi]
        dpm = np.ascontiguousarray(full.reshape(cfg.t, P).T)
        # dstw layout [128, total_tiles]: [p, t] = value of edge slot t*128+p
        dwt = np.ascontiguousarray(dw.reshape(total_tiles, P).T)
        in_maps.append({
            "xT": xT, "dinv_pm": dpm,
            "gidx": _wrap16(gz), "dstw": dwt,
            "W1": np.asarray(W1, np.float32), "W2": np.asarray(W2, np.float32),
            "W3": np.asarray(W3, np.float32), "W4": np.asarray(W4, np.float32),
            "b1": np.asarray(b1, np.float32).reshape(-1, 1),
            "b2": np.asarray(b2, np.float32).reshape(-1, 1),
            "b3": np.asarray(b3, np.float32).reshape(-1, 1),
            "b4": np.asarray(b4, np.float32).reshape(-1, 1),
        })
    return in_maps, plan, total_tiles


# ---------------------------------------------------------------- builder

def build_nc(cfg, plan, total_tiles):
    H = cfg.h
    idx_cols = total_tiles * 8
    nc = bacc.Bacc("TRN2", target_bir_lowering=False, debug=False,
                   num_devices=cfg.nc)
    xT_d = nc.dram_tensor("xT", [cfg.in_feats, cfg.sp], F32, kind="ExternalInput")
    dinv_d = nc.dram_tensor("dinv_pm", [P, cfg.t], F32, kind="ExternalInput")
    gidx_d = nc.dram_tensor("gidx", [P, idx_cols], I16, kind="ExternalInput")
    dstw_d = nc.dram_tensor("dstw", [P, total_tiles], F32, kind="ExternalInput")
    W_d = {w: nc.dram_tensor(w, [cfg.in_feats if w in ("W1", "W4") else H, H],
                             F32, kind="ExternalInput")
           for w in ("W1", "W2", "W3", "W4")}
    b_d = {b: nc.dram_tensor(b, [H, 1], F32, kind="ExternalInput")
           for b in ("b1", "b2", "b3", "b4")}
    outl_d = nc.dram_tensor("out_l", [H, cfg.sp], F32, kind="ExternalOutput")
    outh_d = nc.dram_tensor("out_h", [H, cfg.sp], F32, kind="ExternalOutput")

    relu = mybir.ActivationFunctionType.Relu
    cp = mybir.ActivationFunctionType.Copy

    with tile.TileContext(nc) as tc, ExitStack() as ctx:
        pers = ctx.enter_context(tc.tile_pool(name="pers", bufs=1))
        dram = ctx.enter_context(tc.tile_pool(name="dram", bufs=1, space="DRAM"))
        io = ctx.enter_context(tc.tile_pool(name="io", bufs=2))
        one = ctx.enter_context(tc.tile_pool(name="one", bufs=1))
        idxp = ctx.enter_context(tc.tile_pool(name="idxp", bufs=6))
        gbp = ctx.enter_context(tc.tile_pool(name="gbp", bufs=6))
        gbi = ctx.enter_context(tc.tile_pool(name="gbi", bufs=3))
        psum = ctx.enter_context(tc.tile_pool(name="psum", bufs=2, space="PSUM"))
        psum1 = ctx.enter_context(tc.tile_pool(name="psum1", bufs=2, space="PSUM"))
        psum2 = ctx.enter_context(tc.tile_pool(name="psum2", bufs=2, space="PSUM"))

        nc.gpsimd.load_library(mlp)

        f0 = pers.tile([P, cfg.t, 64], F32, tag="f0")
        f1 = pers.tile([P, cfg.t, 64], F32, tag="f1")
        f2 = pers.tile([P, cfg.t, 64], F32, tag="f2")
        tbl = pers.tile([P, cfg.t + 1, 64], F32, tag="tbl")
        dinv_s = pers.tile([P, cfg.t], F32, tag="dinv")
        Ws = {w: pers.tile([cfg.in_feats if w in ("W1", "W4") else H, H],
                           F32, tag=w, name=w + "_s")
              for w in ("W1", "W2", "W3", "W4")}
        bs = {b: pers.tile([H, 1], F32, tag=b, name=b + "_s")
              for b in ("b1", "b2", "b3", "b4")}
        ident = pers.tile([P, P], F32, tag="ident")
        sid3 = pers.tile([P, P], F32, tag="sid3")
        sid075 = pers.tile([P, P], F32, tag="sid075")
        sidm15 = pers.tile([P, P], F32, tag="sidm15")

        tb_ins = [dram.tile([cfg.tp, 64], F32, name=f"tb_in{r}")
                  for r in range(2)]
        tb_fulls = [dram.tile([cfg.tp * cfg.nc, 64], F32, addr_space="Shared",
                              name=f"tb_full{r}") for r in range(2)]
        agg = pers.tile([P, cfg.t, 64], F32, tag="agg")
        iota_f = pers.tile([P, P], F32, tag="iota_f")

        for w in Ws:
            nc.sync.dma_start(Ws[w][:], W_d[w][:])
        for b in bs:
            nc.sync.dma_start(bs[b][:], b_d[b][:])
        nc.sync.dma_start(dinv_s[:], dinv_d[:])
        make_identity(nc, ident[:])
        nc.vector.tensor_scalar_mul(sid3[:], ident[:], 3.0)
        nc.vector.tensor_scalar_mul(sid075[:], ident[:], 0.75)
        nc.vector.tensor_scalar_mul(sidm15[:], ident[:], -1.5)
        nc.gpsimd.memset(tbl[:, cfg.t, :], 0.0)
        ioti = pers.tile([P, P], mybir.dt.int32, tag="ioti")
        nc.gpsimd.iota(ioti[:], pattern=[[1, P]], base=0, channel_multiplier=0)
        nc.vector.tensor_copy(iota_f[:], ioti[:])

        # ---- phase 1: MLP -> f0 node-major
        CH = cfg.mm_chunk
        for j0 in range(0, cfg.sp, CH):
            w = min(CH, cfg.sp - j0)
            xc = io.tile([cfg.in_feats, CH], F32, tag="xc")
            nc.sync.dma_start(xc[:, :w], xT_d[:, j0:j0 + w])
            ps1 = psum.tile([H, CH], F32, tag="A")
            nc.tensor.matmul(ps1[:, :w], Ws["W1"][:], xc[:, :w],
                             start=True, stop=True)
            h1c = io.tile([H, CH], F32, tag="h1c")
            nc.scalar.activation(h1c[:, :w], ps1[:, :w], relu, bias=bs["b1"][:])
            ps2 = psum.tile([H, CH], F32, tag="B")
            nc.tensor.matmul(ps2[:, :w], Ws["W2"][:], h1c[:, :w],
                             start=True, stop=True)
            h2c = io.tile([H, CH], F32, tag="h2c")
            nc.scalar.activation(h2c[:, :w], ps2[:, :w], relu, bias=bs["b2"][:])
            for i in range(w // P):
                t = (j0 + i * P) // P
                ps3 = psum1.tile([P, 64], F32, tag="C")
                nc.tensor.transpose(ps3[:], h2c[:, i * P:(i + 1) * P],
                                    ident[:H, :H])
                nc.scalar.activation(f0[:, t, :], ps3[:], cp)

        # ---- message passing rounds
        for rnd, (fprev, fnext) in enumerate([(f0, f1), (f1, f2)]):
            tb_in, tb_full = tb_ins[rnd], tb_fulls[rnd]
            nc.vector.tensor_tensor(
                tbl[:, :cfg.t, :], fprev[:],
                dinv_s[:, :, None].to_broadcast([P, cfg.t, 64]),
                mybir.AluOpType.mult)
            nc.sync.dma_start(
                tb_in[:].rearrange("(t p) f -> p t f", p=P), tbl[:])
            nc.gpsimd.collective_compute(
                "AllGather", mybir.AluOpType.bypass,
                replica_groups=[list(range(cfg.nc))],
                ins=[tb_in[:]], outs=[tb_full[:]])
            nc.gpsimd.memset(agg[:], 0.0)
            goff = 0
            for (c, btiles, spans) in plan:
                gi = idxp.tile([P, cfg.max_span_tiles * 8], I16, tag="gi")
                dwv = idxp.tile([P, cfg.max_span_tiles], F32, tag="dwv")
                nc.sync.dma_start(gi[:, :btiles * 8],
                                  gidx_d[:, goff * 8:(goff + btiles) * 8])
                nc.sync.dma_start(dwv[:, :btiles],
                                  dstw_d[:, goff:goff + btiles])
                gb = gbp.tile([P, cfg.max_span_tiles, 64], F32, tag="gb")
                ni = btiles * P
                nc.gpsimd.dma_gather(
                    gb[:, :btiles, :],
                    tb_full[c * cfg.chunk:(c + 1) * cfg.chunk, :],
                    gi[:, :btiles * 8], ni, ni, 64)
                ind = gbi.tile([P, cfg.max_span_tiles, P], F32, tag="ind")
                nc.vector.tensor_tensor(
                    ind[:, :btiles, :],
                    iota_f[:, None, :].to_broadcast([P, btiles, P]),
                    dwv[:, :btiles, None].to_broadcast([P, btiles, P]),
                    mybir.AluOpType.is_equal)
                for (toff, nt, ww) in spans:
                    pw = psum2.tile([P, 64], F32, tag="D")
                    for i in range(nt):
                        nc.tensor.matmul(pw[:], ind[:, toff + i, :],
                                         gb[:, toff + i, :],
                                         start=(i == 0), stop=(i == nt - 1))
                    nc.vector.tensor_tensor(agg[:, ww, :], agg[:, ww, :],
                                            pw[:], mybir.AluOpType.add)
                goff += btiles
            nc.vector.tensor_tensor(
                tbl[:, :cfg.t, :], agg[:],
                dinv_s[:, :, None].to_broadcast([P, cfg.t, 64]),
                mybir.AluOpType.mult)
            nc.vector.tensor_tensor(fnext[:], fprev[:], tbl[:, :cfg.t, :],
                                    mybir.AluOpType.subtract)

        # ---- filters + output MLPs
        nc.vector.tensor_tensor(f0[:], f0[:], f1[:], mybir.AluOpType.subtract)
        for j0 in range(0, cfg.sp, CH):
            w = min(CH, cfg.sp - j0)
            zl = psum.tile([H, CH], F32, tag="A")
            z1 = psum.tile([H, CH], F32, tag="B")
            z2 = psum1.tile([H, CH], F32, tag="C")
            for i in range(w // P):
                t = (j0 + i * P) // P
                cs = slice(i * P, (i + 1) * P)
                nc.tensor.matmul(zl[:, cs], f0[:, t, :], sid3[:],
                                 start=True, stop=False)
                nc.tensor.matmul(zl[:, cs], f2[:, t, :], sid075[:],
                                 start=False, stop=True)
                nc.tensor.matmul(z1[:, cs], f1[:, t, :], sid3[:],
                                 start=True, stop=False)
                nc.tensor.matmul(z1[:, cs], f2[:, t, :], sidm15[:],
                                 start=False, stop=True)
                nc.tensor.matmul(z2[:, cs], f2[:, t, :], sid075[:],
                                 start=True, stop=True)
            zlc = io.tile([H, CH], F32, tag="zlc")
            zhc = io.tile([P, CH], F32, tag="zhc")
            nc.scalar.activation(zlc[:, :w], zl[:, :w], cp)
            nc.scalar.activation(zhc[:H, :w], z1[:, :w], cp)
            nc.scalar.activation(zhc[H:, :w], z2[:, :w], cp)
            pl = psum1.tile([H, CH], F32, tag="C")
            ph = psum.tile([H, CH], F32, tag="A")
            nc.tensor.matmul(pl[:, :w], Ws["W3"][:], zlc[:, :w],
                             start=True, stop=True)
            nc.tensor.matmul(ph[:, :w], Ws["W4"][:], zhc[:, :w],
                             start=True, stop=True)
            ol = io.tile([H, CH], F32, tag="ol")
            oh = io.tile([H, CH], F32, tag="oh")
            nc.scalar.activation(ol[:, :w], pl[:, :w], relu, bias=bs["b3"][:])
            nc.scalar.activation(oh[:, :w], ph[:, :w], relu, bias=bs["b4"][:])
            nc.sync.dma_start(outl_d[:, j0:j0 + w], ol[:, :w])
            nc.sync.dma_start(outh_d[:, j0:j0 + w], oh[:, :w])

    nc.compile()
    return nc


# ---------------------------------------------------------------- driver

_CACHE = {}


def run(cfg, inputs, run_fn=None, **spmd_kwargs):
    in_maps, plan, total_tiles = preprocess(cfg, **inputs)
    key = (cfg.n_nodes, cfg.n_edges, total_tiles,
           tuple((c, b, tuple(s)) for c, b, s in plan))
    if key not in _CACHE:
        _CACHE[key] = build_nc(cfg, plan, total_tiles)
    nc = _CACHE[key]
    if run_fn is not None:
        results = run_fn(nc, in_maps)
        res = None
    else:
        res = run_bass_kernel_spmd(nc, in_maps, core_ids=list(range(cfg.nc)), **spmd_kwargs)
        results = res.results
    h_l = np.zeros((cfg.n_nodes, cfg.h), np.float32)
    h_h = np.zeros((cfg.n_nodes, cfg.h), np.float32)
    for c in range(cfg.nc):
        lo = c * cfg.shard
        h_l[lo:lo + cfg.shard] = results[c]["out_l"].T[:cfg.shard]
        h_h[lo:lo + cfg.shard] = results[c]["out_h"].T[:cfg.shard]
    return h_l, h_h, res


def kernel(in_feat, src, dst, W1, b1, W2, b2, W3, b3, W4, b4):
    cfg = Cfg(100000, 1600000, 128, 64, 8)
    h_l, h_h, _ = run(cfg, dict(
        in_feat=np.asarray(in_feat, np.float32),
        src=np.asarray(src, np.int64), dst=np.asarray(dst, np.int64),
        W1=np.asarray(W1, np.float32), b1=np.asarray(b1, np.float32),
        W2=np.asarray(W2, np.float32), b2=np.asarray(b2, np.float32),
        W3=np.asarray(W3, np.float32), b3=np.asarray(b3, np.float32),
        W4=np.asarray(W4, np.float32), b4=np.asarray(b4, np.float32)))
    return h_l, h_h


# revision 16
# speedup vs baseline: 1.7713x; 1.0013x over previous
"""BWGNN (Bernstein-wavelet GNN) Trainium2 kernel, 8-core SPMD.

Sharding: nodes split 8 ways (graph/data parallel); edges partitioned by dst
shard; tiny weights replicated.  Per round of Laplacian message passing the
node-state table (dinv * f) is AllGathered, then per-edge src rows are
fetched with dma_gather (int16 indices -> the global table is addressed in
<=25344-row chunks) and segment-summed by dst via dma_scatter_add.  A
scatter instruction must not contain two edges with the same dst (the SDMA
CCE read-modify-write races on duplicates - measured on HW), so edges are
"layered": within a (src-chunk, dst) group, edge #k goes to layer k; every
scatter span stays inside one layer.  Spans rotate over NBUF DRAM agg
buffers (Tile WAW serializes per buffer; buffers overlap), summed on-chip.

MLP in/out runs feature-major with stationary-weight matmuls; node-major
states are produced by PE transposes; the three Bernstein filters are fused
into the transposes with scaled identity matrices.  Outputs are written
feature-major [64, shard]; the host transposes and concatenates.
"""

import sys
from contextlib import ExitStack

import numpy as np

try:
    import concourse  # noqa: F401
except ImportError:  # pragma: no cover
    sys.path.insert(0, "/opt/trn_rl_repo")

import concourse.bacc as bacc
import concourse.bass as bass
import concourse.mybir as mybir
import concourse.tile as tile
from concourse.bass_utils import run_bass_kernel_spmd
from concourse.library_config import mlp
from concourse.masks import make_identity

P = 128
F32 = mybir.dt.float32
I16 = mybir.dt.int16


class Cfg:
    def __init__(self, n_nodes, n_edges, in_feats, h_feats, n_cores,
                 max_span_tiles=8, nbuf=4, mm_chunk=512):
        assert n_nodes % n_cores == 0
        self.n_nodes, self.n_edges = n_nodes, n_edges
        self.in_feats, self.h = in_feats, h_feats
        self.nc = n_cores
        self.shard = n_nodes // n_cores
        self.sp = ((self.shard + P - 1) // P) * P      # padded shard
        self.t = self.sp // P                          # node tiles
        self.tp = self.sp + P                          # table rows/core
        self.tbl_rows = self.tp * n_cores
        self.n_chunks = max(1, (self.tbl_rows + 25343) // 25344)
        self.chunk_shards = -(-n_cores // self.n_chunks)  # shards per chunk
        self.chunk = self.chunk_shards * self.tp
        assert self.chunk <= 32640, self.chunk
        self.n_chunks = -(-n_cores // self.chunk_shards)
        self.max_span_tiles = max_span_tiles           # per-instr tile cap
        self.nbuf = nbuf
        self.mm_chunk = mm_chunk


# ---------------------------------------------------------------- host prep

def _per_core_groups(cfg, srcrow, dstloc):
    """-> dict (chunk, window) -> (src_local_i16[], dst_in_window_f32[])."""
    q = srcrow // cfg.chunk
    out = {}
    for c in range(cfg.n_chunks):
        m = q == c
        sc = (srcrow[m] - c * cfg.chunk).astype(np.int64)
        dc = dstloc[m].astype(np.int64)
        o = np.argsort(dc, kind="stable")
        sc, dc = sc[o], dc[o]
        w = dc // P
        for ww in np.unique(w):
            sel = w == ww
            out[(c, int(ww))] = (sc[sel].astype(np.int16),
                                 (dc[sel] % P).astype(np.float32))
    return out


def _wrap16(x):
    """flat int16 stream -> [128, n/16]: storage[p, col] = x[col*16 + p%16]."""
    assert len(x) % 16 == 0
    return np.tile(x.reshape(-1, 16).T, (8, 1)).copy()


def preprocess(cfg, in_feat, src, dst, W1, b1, W2, b2, W3, b3, W4, b4):
    n = cfg.n_nodes
    deg = np.bincount(dst, minlength=n).astype(np.float32)
    dinv = np.clip(deg, 1.0, None) ** -0.5

    shard_of = dst // cfg.shard
    groups = []
    for c in range(cfg.nc):
        m = shard_of == c
        src_c, dst_c = src[m].astype(np.int64), dst[m].astype(np.int64)
        srcrow = (src_c // cfg.shard) * cfg.tp + (src_c % cfg.shard)
        dstloc = dst_c - c * cfg.shard
        groups.append(_per_core_groups(cfg, srcrow, dstloc))

    # canonical (chunk, window) sizes = max over cores, cut into spans
    keys = sorted(set().union(*[set(g.keys()) for g in groups]))
    plan = []  # bundles: (chunk, btiles, [(toff, ntiles, window)])
    cur_chunk, cur_spans, cur_tiles = None, [], 0

    def flush():
        nonlocal cur_spans, cur_tiles
        if cur_spans:
            plan.append((cur_chunk, cur_tiles, cur_spans))
        cur_spans, cur_tiles = [], 0

    span_src = []  # (chunk, window, pos, ntiles) in stream order
    for (c, ww) in keys:
        size = max(len(g.get((c, ww), ((), ()))[0]) for g in groups)
        pos = 0
        while pos < size:
            take = min(cfg.max_span_tiles * P, size - pos)
            nt = -(-take // P)
            if cur_chunk != c or cur_tiles + nt > cfg.max_span_tiles:
                flush()
                cur_chunk = c
            cur_spans.append((cur_tiles, nt, ww))
            span_src.append((c, ww, pos, nt))
            cur_tiles += nt
            pos += take
    flush()

    total_tiles = sum(b[1] for b in plan)
    in_maps = []
    zero_local = [s * cfg.tp + cfg.sp for s in range(cfg.nc)]

    def chunk_zero(c):
        for z in zero_local:
            if c * cfg.chunk <= z < (c + 1) * cfg.chunk:
                return z - c * cfg.chunk
        raise AssertionError("no zero row in chunk")

    for core in range(cfg.nc):
        gz = np.zeros(total_tiles * P, np.int16)
        dw = np.full(total_tiles * P, 999.0, np.float32)
        base = 0
        si = 0
        for (c, btiles, spans) in plan:
            for (toff, nt, ww) in spans:
                (cc, ww2, pos, nt2) = span_src[si]
                si += 1
                assert cc == c and nt2 == nt and ww2 == ww
                s_arr, d_arr = groups[core].get(
                    (c, ww), (np.zeros(0, np.int16), np.zeros(0, np.float32)))
                seg_s = s_arr[pos:pos + nt * P]
                seg_d = d_arr[pos:pos + nt * P]
                o = base + toff * P
                gz[o:o + nt * P] = chunk_zero(c)
                gz[o:o + len(seg_s)] = seg_s
                dw[o:o + len(seg_d)] = seg_d
            base += btiles * P
        lo, hi = core * cfg.shard, (core + 1) * cfg.shard
        xT = np.zeros((cfg.in_feats, cfg.sp), np.float32)
        xT[:, :cfg.shard] = in_feat[lo:hi].T
        full = np.ones(cfg.sp, np.float32)
        full[:cfg.shard] = dinv[lo:hi]
        dpm = np.ascontiguousarray(full.reshape(cfg.t, P).T)
        # dstw layout [128, total_tiles]: [p, t] = value of edge slot t*128+p
        dwt = np.ascontiguousarray(dw.reshape(total_tiles, P).T)
        in_maps.append({
            "xT": xT, "dinv_pm": dpm,
            "gidx": _wrap16(gz), "dstw": dwt,
            "W1": np.asarray(W1, np.float32), "W2": np.asarray(W2, np.float32),
            "W3": np.asarray(W3, np.float32), "W4": np.asarray(W4, np.float32),
            "b1": np.asarray(b1, np.float32).reshape(-1, 1),
            "b2": np.asarray(b2, np.float32).reshape(-1, 1),
            "b3": np.asarray(b3, np.float32).reshape(-1, 1),
            "b4": np.asarray(b4, np.float32).reshape(-1, 1),
        })
    return in_maps, plan, total_tiles


# ---------------------------------------------------------------- builder

def build_nc(cfg, plan, total_tiles):
    H = cfg.h
    idx_cols = total_tiles * 8
    nc = bacc.Bacc("TRN2", target_bir_lowering=False, debug=False,
                   num_devices=cfg.nc)
    xT_d = nc.dram_tensor("xT", [cfg.in_feats, cfg.sp], F32, kind="ExternalInput")
    dinv_d = nc.dram_tensor("dinv_pm", [P, cfg.t], F32, kind="ExternalInput")
    gidx_d = nc.dram_tensor("gidx", [P, idx_cols], I16, kind="ExternalInput")
    dstw_d = nc.dram_tensor("dstw", [P, total_tiles], F32, kind="ExternalInput")
    W_d = {w: nc.dram_tensor(w, [cfg.in_feats if w in ("W1", "W4") else H, H],
                             F32, kind="ExternalInput")
           for w in ("W1", "W2", "W3", "W4")}
    b_d = {b: nc.dram_tensor(b, [H, 1], F32, kind="ExternalInput")
           for b in ("b1", "b2", "b3", "b4")}
    outl_d = nc.dram_tensor("out_l", [H, cfg.sp], F32, kind="ExternalOutput")
    outh_d = nc.dram_tensor("out_h", [H, cfg.sp], F32, kind="ExternalOutput")

    relu = mybir.ActivationFunctionType.Relu
    cp = mybir.ActivationFunctionType.Copy

    with tile.TileContext(nc) as tc, ExitStack() as ctx:
        pers = ctx.enter_context(tc.tile_pool(name="pers", bufs=1))
        dram = ctx.enter_context(tc.tile_pool(name="dram", bufs=1, space="DRAM"))
        io = ctx.enter_context(tc.tile_pool(name="io", bufs=2))
        one = ctx.enter_context(tc.tile_pool(name="one", bufs=1))
        idxp = ctx.enter_context(tc.tile_pool(name="idxp", bufs=6))
        gbp = ctx.enter_context(tc.tile_pool(name="gbp", bufs=6))
        gbi = ctx.enter_context(tc.tile_pool(name="gbi", bufs=3))
        psum = ctx.enter_context(tc.tile_pool(name="psum", bufs=2, space="PSUM"))
        psum1 = ctx.enter_context(tc.tile_pool(name="psum1", bufs=2, space="PSUM"))
        psum2 = ctx.enter_context(tc.tile_pool(name="psum2", bufs=2, space="PSUM"))

        nc.gpsimd.load_library(mlp)

        f0 = pers.tile([P, cfg.t, 64], F32, tag="f0")
        f1 = pers.tile([P, cfg.t, 64], F32, tag="f1")
        f2 = pers.tile([P, cfg.t, 64], F32, tag="f2")
        tbl = pers.tile([P, cfg.t + 1, 64], F32, tag="tbl")
        dinv_s = pers.tile([P, cfg.t], F32, tag="dinv")
        Ws = {w: pers.tile([cfg.in_feats if w in ("W1", "W4") else H, H],
                           F32, tag=w, name=w + "_s")
              for w in ("W1", "W2", "W3", "W4")}
        bs = {b: pers.tile([H, 1], F32, tag=b, name=b + "_s")
              for b in ("b1", "b2", "b3", "b4")}
        ident = pers.tile([P, P], F32, tag="ident")
        sid3 = pers.tile([P, P], F32, tag="sid3")
        sid075 = pers.tile([P, P], F32, tag="sid075")
        sidm15 = pers.tile([P, P], F32, tag="sidm15")

        tb_ins = [dram.tile([cfg.tp, 64], F32, name=f"tb_in{r}")
                  for r in range(2)]
        tb_fulls = [dram.tile([cfg.tp * cfg.nc, 64], F32, addr_space="Shared",
                              name=f"tb_full{r}") for r in range(2)]
        agg = pers.tile([P, cfg.t, 64], F32, tag="agg")
        iota_f = pers.tile([P, P], F32, tag="iota_f")

        for w in Ws:
            nc.sync.dma_start(Ws[w][:], W_d[w][:])
        for b in bs:
            nc.sync.dma_start(bs[b][:], b_d[b][:])
        nc.sync.dma_start(dinv_s[:], dinv_d[:])
        make_identity(nc, ident[:])
        nc.vector.tensor_scalar_mul(sid3[:], ident[:], 3.0)
        nc.vector.tensor_scalar_mul(sid075[:], ident[:], 0.75)
        nc.vector.tensor_scalar_mul(sidm15[:], ident[:], -1.5)
        nc.gpsimd.memset(tbl[:, cfg.t, :], 0.0)
        ioti = pers.tile([P, P], mybir.dt.int32, tag="ioti")
        nc.gpsimd.iota(ioti[:], pattern=[[1, P]], base=0, channel_multiplier=0)
        nc.vector.tensor_copy(iota_f[:], ioti[:])

        # ---- phase 1: MLP -> f0 node-major
        CH = cfg.mm_chunk
        for j0 in range(0, cfg.sp, CH):
            w = min(CH, cfg.sp - j0)
            xc = io.tile([cfg.in_feats, CH], F32, tag="xc")
            nc.sync.dma_start(xc[:, :w], xT_d[:, j0:j0 + w])
            ps1 = psum.tile([H, CH], F32, tag="A")
            nc.tensor.matmul(ps1[:, :w], Ws["W1"][:], xc[:, :w],
                             start=True, stop=True)
            h1c = io.tile([H, CH], F32, tag="h1c")
            nc.scalar.activation(h1c[:, :w], ps1[:, :w], relu, bias=bs["b1"][:])
            ps2 = psum.tile([H, CH], F32, tag="B")
            nc.tensor.matmul(ps2[:, :w], Ws["W2"][:], h1c[:, :w],
                             start=True, stop=True)
            h2c = io.tile([H, CH], F32, tag="h2c")
            nc.scalar.activation(h2c[:, :w], ps2[:, :w], relu, bias=bs["b2"][:])
            for i in range(w // P):
                t = (j0 + i * P) // P
                ps3 = psum1.tile([P, 64], F32, tag="C")
                nc.tensor.transpose(ps3[:], h2c[:, i * P:(i + 1) * P],
                                    ident[:H, :H])
                nc.scalar.activation(f0[:, t, :], ps3[:], cp)

        # ---- message passing rounds
        for rnd, (fprev, fnext) in enumerate([(f0, f1), (f1, f2)]):
            tb_in, tb_full = tb_ins[rnd], tb_fulls[rnd]
            nc.vector.tensor_tensor(
                tbl[:, :cfg.t, :], fprev[:],
                dinv_s[:, :, None].to_broadcast([P, cfg.t, 64]),
                mybir.AluOpType.mult)
            nc.sync.dma_start(
                tb_in[:].rearrange("(t p) f -> p t f", p=P), tbl[:])
            nc.gpsimd.collective_compute(
                "AllGather", mybir.AluOpType.bypass,
                replica_groups=[list(range(cfg.nc))],
                ins=[tb_in[:]], outs=[tb_full[:]])
            nc.vector.tensor_scalar_mul(agg[:], agg[:], 0.0)
            goff = 0
            for (c, btiles, spans) in plan:
                gi = idxp.tile([P, cfg.max_span_tiles * 8], I16, tag="gi")
                dwv = idxp.tile([P, cfg.max_span_tiles], F32, tag="dwv")
                nc.sync.dma_start(gi[:, :btiles * 8],
                                  gidx_d[:, goff * 8:(goff + btiles) * 8])
                nc.sync.dma_start(dwv[:, :btiles],
                                  dstw_d[:, goff:goff + btiles])
                gb = gbp.tile([P, cfg.max_span_tiles, 64], F32, tag="gb")
                ni = btiles * P
                nc.gpsimd.dma_gather(
                    gb[:, :btiles, :],
                    tb_full[c * cfg.chunk:(c + 1) * cfg.chunk, :],
                    gi[:, :btiles * 8], ni, ni, 64)
                ind = gbi.tile([P, cfg.max_span_tiles, P], F32, tag="ind")
                nc.vector.tensor_tensor(
                    ind[:, :btiles, :],
                    iota_f[:, None, :].to_broadcast([P, btiles, P]),
                    dwv[:, :btiles, None].to_broadcast([P, btiles, P]),
                    mybir.AluOpType.is_equal)
                for (toff, nt, ww) in spans:
                    pw = psum2.tile([P, 64], F32, tag="D")
                    for i in range(nt):
                        nc.tensor.matmul(pw[:], ind[:, toff + i, :],
                                         gb[:, toff + i, :],
                                         start=(i == 0), stop=(i == nt - 1))
                    nc.vector.tensor_tensor(agg[:, ww, :], agg[:, ww, :],
                                            pw[:], mybir.AluOpType.add)
                goff += btiles
            nc.vector.tensor_tensor(
                tbl[:, :cfg.t, :], agg[:],
                dinv_s[:, :, None].to_broadcast([P, cfg.t, 64]),
                mybir.AluOpType.mult)
            nc.vector.tensor_tensor(fnext[:], fprev[:], tbl[:, :cfg.t, :],
                                    mybir.AluOpType.subtract)

        # ---- filters + output MLPs
        nc.vector.tensor_tensor(f0[:], f0[:], f1[:], mybir.AluOpType.subtract)
        for j0 in range(0, cfg.sp, CH):
            w = min(CH, cfg.sp - j0)
            zl = psum.tile([H, CH], F32, tag="A")
            z1 = psum.tile([H, CH], F32, tag="B")
            z2 = psum1.tile([H, CH], F32, tag="C")
            for i in range(w // P):
                t = (j0 + i * P) // P
                cs = slice(i * P, (i + 1) * P)
                nc.tensor.matmul(zl[:, cs], f0[:, t, :], sid3[:],
                                 start=True, stop=False)
                nc.tensor.matmul(zl[:, cs], f2[:, t, :], sid075[:],
                                 start=False, stop=True)
                nc.tensor.matmul(z1[:, cs], f1[:, t, :], sid3[:],
                                 start=True, stop=False)
                nc.tensor.matmul(z1[:, cs], f2[:, t, :], sidm15[:],
                                 start=False, stop=True)
                nc.tensor.matmul(z2[:, cs], f2[:, t, :], sid075[:],
                                 start=True, stop=True)
            zlc = io.tile([H, CH], F32, tag="zlc")
            zhc = io.tile([P, CH], F32, tag="zhc")
            nc.scalar.activation(zlc[:, :w], zl[:, :w], cp)
            nc.scalar.activation(zhc[:H, :w], z1[:, :w], cp)
            nc.scalar.activation(zhc[H:, :w], z2[:, :w], cp)
            pl = psum1.tile([H, CH], F32, tag="C")
            ph = psum.tile([H, CH], F32, tag="A")
            nc.tensor.matmul(pl[:, :w], Ws["W3"][:], zlc[:, :w],
                             start=True, stop=True)
            nc.tensor.matmul(ph[:, :w], Ws["W4"][:], zhc[:, :w],
                             start=True, stop=True)
            ol = io.tile([H, CH], F32, tag="ol")
            oh = io.tile([H, CH], F32, tag="oh")
            nc.scalar.activation(ol[:, :w], pl[:, :w], relu, bias=bs["b3"][:])
            nc.scalar.activation(oh[:, :w], ph[:, :w], relu, bias=bs["b4"][:])
            nc.sync.dma_start(outl_d[:, j0:j0 + w], ol[:, :w])
            nc.sync.dma_start(outh_d[:, j0:j0 + w], oh[:, :w])

    nc.compile()
    return nc


# ---------------------------------------------------------------- driver

_CACHE = {}


def run(cfg, inputs, run_fn=None, **spmd_kwargs):
    in_maps, plan, total_tiles = preprocess(cfg, **inputs)
    key = (cfg.n_nodes, cfg.n_edges, total_tiles,
           tuple((c, b, tuple(s)) for c, b, s in plan))
    if key not in _CACHE:
        _CACHE[key] = build_nc(cfg, plan, total_tiles)
    nc = _CACHE[key]
    if run_fn is not None:
        results = run_fn(nc, in_maps)
        res = None
    else:
        res = run_bass_kernel_spmd(nc, in_maps, core_ids=list(range(cfg.nc)), **spmd_kwargs)
        results = res.results
    h_l = np.zeros((cfg.n_nodes, cfg.h), np.float32)
    h_h = np.zeros((cfg.n_nodes, cfg.h), np.float32)
    for c in range(cfg.nc):
        lo = c * cfg.shard
        h_l[lo:lo + cfg.shard] = results[c]["out_l"].T[:cfg.shard]
        h_h[lo:lo + cfg.shard] = results[c]["out_h"].T[:cfg.shard]
    return h_l, h_h, res


def kernel(in_feat, src, dst, W1, b1, W2, b2, W3, b3, W4, b4):
    cfg = Cfg(100000, 1600000, 128, 64, 8)
    h_l, h_h, _ = run(cfg, dict(
        in_feat=np.asarray(in_feat, np.float32),
        src=np.asarray(src, np.int64), dst=np.asarray(dst, np.int64),
        W1=np.asarray(W1, np.float32), b1=np.asarray(b1, np.float32),
        W2=np.asarray(W2, np.float32), b2=np.asarray(b2, np.float32),
        W3=np.asarray(W3, np.float32), b3=np.asarray(b3, np.float32),
        W4=np.asarray(W4, np.float32), b4=np.asarray(b4, np.float32)))
    return h_l, h_h


# revision 19
# speedup vs baseline: 1.8158x; 1.0252x over previous
"""BWGNN (Bernstein-wavelet GNN) Trainium2 kernel, 8-core SPMD.

Sharding: nodes split 8 ways (graph/data parallel); edges partitioned by dst
shard; tiny weights replicated.  Per round of Laplacian message passing the
node-state table (dinv * f) is AllGathered, then per-edge src rows are
fetched with dma_gather (int16 indices -> the global table is addressed in
<=25344-row chunks; <=1024 indices per instruction, the HW SWDGE ring cap).

Segment-sum by dst runs on the TensorEngine: edges are sorted by (src-chunk,
dst-window-of-128); per 128-edge tile the VectorEngine builds a one-hot
indicator [128e, 128dst] (iota vs dst%128, both broadcast via step-0 APs),
and matmuls accumulate each window in PSUM, added into an SBUF-resident agg.
This avoids dma_scatter_add entirely (its CCE read-modify-write races on
duplicate indices, and its descriptor generation was half the POOL time).

MLP in/out runs feature-major with stationary-weight matmuls; node-major
states are produced by PE transposes; the three Bernstein filters are fused
into the transposes with scaled identity matrices.  Outputs are written
feature-major [64, shard]; the host transposes and concatenates.
Measured: 5.02 ms exec (neuron-profile), rel err 6.5e-7 vs the jax reference.
"""

import sys
from contextlib import ExitStack

import numpy as np

try:
    import concourse  # noqa: F401
except ImportError:  # pragma: no cover
    sys.path.insert(0, "/opt/trn_rl_repo")

import concourse.bacc as bacc
import concourse.bass as bass
import concourse.mybir as mybir
import concourse.tile as tile
from concourse.bass_utils import run_bass_kernel_spmd
from concourse.library_config import mlp
from concourse.masks import make_identity

P = 128
F32 = mybir.dt.float32
I16 = mybir.dt.int16


class Cfg:
    def __init__(self, n_nodes, n_edges, in_feats, h_feats, n_cores,
                 max_span_tiles=8, nbuf=4, mm_chunk=512):
        assert n_nodes % n_cores == 0
        self.n_nodes, self.n_edges = n_nodes, n_edges
        self.in_feats, self.h = in_feats, h_feats
        self.nc = n_cores
        self.shard = n_nodes // n_cores
        self.sp = ((self.shard + P - 1) // P) * P      # padded shard
        self.t = self.sp // P                          # node tiles
        self.tp = self.sp + P                          # table rows/core
        self.tbl_rows = self.tp * n_cores
        self.n_chunks = max(1, (self.tbl_rows + 25343) // 25344)
        self.chunk_shards = -(-n_cores // self.n_chunks)  # shards per chunk
        self.chunk = self.chunk_shards * self.tp
        assert self.chunk <= 32640, self.chunk
        self.n_chunks = -(-n_cores // self.chunk_shards)
        self.max_span_tiles = max_span_tiles           # per-instr tile cap
        self.nbuf = nbuf
        self.mm_chunk = mm_chunk


# ---------------------------------------------------------------- host prep

def _per_core_groups(cfg, srcrow, dstloc):
    """-> dict (chunk, window) -> (src_local_i16[], dst_in_window_f32[])."""
    q = srcrow // cfg.chunk
    out = {}
    for c in range(cfg.n_chunks):
        m = q == c
        sc = (srcrow[m] - c * cfg.chunk).astype(np.int64)
        dc = dstloc[m].astype(np.int64)
        o = np.argsort(dc, kind="stable")
        sc, dc = sc[o], dc[o]
        w = dc // P
        for ww in np.unique(w):
            sel = w == ww
            out[(c, int(ww))] = (sc[sel].astype(np.int16),
                                 (dc[sel] % P).astype(np.float32))
    return out


def _wrap16(x):
    """flat int16 stream -> [128, n/16]: storage[p, col] = x[col*16 + p%16]."""
    assert len(x) % 16 == 0
    return np.tile(x.reshape(-1, 16).T, (8, 1)).copy()


def preprocess(cfg, in_feat, src, dst, W1, b1, W2, b2, W3, b3, W4, b4):
    n = cfg.n_nodes
    deg = np.bincount(dst, minlength=n).astype(np.float32)
    dinv = np.clip(deg, 1.0, None) ** -0.5

    shard_of = dst // cfg.shard
    groups = []
    for c in range(cfg.nc):
        m = shard_of == c
        src_c, dst_c = src[m].astype(np.int64), dst[m].astype(np.int64)
        srcrow = (src_c // cfg.shard) * cfg.tp + (src_c % cfg.shard)
        dstloc = dst_c - c * cfg.shard
        groups.append(_per_core_groups(cfg, srcrow, dstloc))

    # canonical (chunk, window) sizes = max over cores, cut into spans
    keys = sorted(set().union(*[set(g.keys()) for g in groups]))
    plan = []  # bundles: (chunk, btiles, [(toff, ntiles, window)])
    cur_chunk, cur_spans, cur_tiles = None, [], 0

    def flush():
        nonlocal cur_spans, cur_tiles
        if cur_spans:
            plan.append((cur_chunk, cur_tiles, cur_spans))
        cur_spans, cur_tiles = [], 0

    span_src = []  # (chunk, window, pos, ntiles) in stream order
    for (c, ww) in keys:
        size = max(len(g.get((c, ww), ((), ()))[0]) for g in groups)
        gtiles = -(-size // P)
        tpos = 0
        while tpos < gtiles:
            if cur_chunk != c:
                flush()
                cur_chunk = c
            room = cfg.max_span_tiles - cur_tiles
            if room == 0:
                flush()
                room = cfg.max_span_tiles
            nt = min(room, gtiles - tpos)
            cur_spans.append((cur_tiles, nt, ww))
            span_src.append((c, ww, tpos * P, nt))
            cur_tiles += nt
            tpos += nt
    flush()

    total_tiles = sum(b[1] for b in plan)
    in_maps = []
    zero_local = [s * cfg.tp + cfg.sp for s in range(cfg.nc)]

    def chunk_zero(c):
        for z in zero_local:
            if c * cfg.chunk <= z < (c + 1) * cfg.chunk:
                return z - c * cfg.chunk
        raise AssertionError("no zero row in chunk")

    for core in range(cfg.nc):
        gz = np.zeros(total_tiles * P, np.int16)
        dw = np.full(total_tiles * P, 999.0, np.float32)
        base = 0
        si = 0
        for (c, btiles, spans) in plan:
            for (toff, nt, ww) in spans:
                (cc, ww2, pos, nt2) = span_src[si]
                si += 1
                assert cc == c and nt2 == nt and ww2 == ww
                s_arr, d_arr = groups[core].get(
                    (c, ww), (np.zeros(0, np.int16), np.zeros(0, np.float32)))
                seg_s = s_arr[pos:pos + nt * P]
                seg_d = d_arr[pos:pos + nt * P]
                o = base + toff * P
                gz[o:o + nt * P] = chunk_zero(c)
                gz[o:o + len(seg_s)] = seg_s
                dw[o:o + len(seg_d)] = seg_d
            base += btiles * P
        lo, hi = core * cfg.shard, (core + 1) * cfg.shard
        xT = np.zeros((cfg.in_feats, cfg.sp), np.float32)
        xT[:, :cfg.shard] = in_feat[lo:hi].T
        full = np.ones(cfg.sp, np.float32)
        full[:cfg.shard] = dinv[lo:hi]
        dpm = np.ascontiguousarray(full.reshape(cfg.t, P).T)
        # dstw layout [128, total_tiles]: [p, t] = value of edge slot t*128+p
        dwt = np.ascontiguousarray(dw.reshape(total_tiles, P).T)
        in_maps.append({
            "xT": xT, "dinv_pm": dpm,
            "gidx": _wrap16(gz), "dstw": dwt,
            "W1": np.asarray(W1, np.float32), "W2": np.asarray(W2, np.float32),
            "W3": np.asarray(W3, np.float32), "W4": np.asarray(W4, np.float32),
            "b1": np.asarray(b1, np.float32).reshape(-1, 1),
            "b2": np.asarray(b2, np.float32).reshape(-1, 1),
            "b3": np.asarray(b3, np.float32).reshape(-1, 1),
            "b4": np.asarray(b4, np.float32).reshape(-1, 1),
        })
    return in_maps, plan, total_tiles


# ---------------------------------------------------------------- builder

def build_nc(cfg, plan, total_tiles):
    H = cfg.h
    idx_cols = total_tiles * 8
    nc = bacc.Bacc("TRN2", target_bir_lowering=False, debug=False,
                   num_devices=cfg.nc)
    xT_d = nc.dram_tensor("xT", [cfg.in_feats, cfg.sp], F32, kind="ExternalInput")
    dinv_d = nc.dram_tensor("dinv_pm", [P, cfg.t], F32, kind="ExternalInput")
    gidx_d = nc.dram_tensor("gidx", [P, idx_cols], I16, kind="ExternalInput")
    dstw_d = nc.dram_tensor("dstw", [P, total_tiles], F32, kind="ExternalInput")
    W_d = {w: nc.dram_tensor(w, [cfg.in_feats if w in ("W1", "W4") else H, H],
                             F32, kind="ExternalInput")
           for w in ("W1", "W2", "W3", "W4")}
    b_d = {b: nc.dram_tensor(b, [H, 1], F32, kind="ExternalInput")
           for b in ("b1", "b2", "b3", "b4")}
    outl_d = nc.dram_tensor("out_l", [H, cfg.sp], F32, kind="ExternalOutput")
    outh_d = nc.dram_tensor("out_h", [H, cfg.sp], F32, kind="ExternalOutput")

    relu = mybir.ActivationFunctionType.Relu
    cp = mybir.ActivationFunctionType.Copy

    with tile.TileContext(nc) as tc, ExitStack() as ctx:
        pers = ctx.enter_context(tc.tile_pool(name="pers", bufs=1))
        dram = ctx.enter_context(tc.tile_pool(name="dram", bufs=1, space="DRAM"))
        io = ctx.enter_context(tc.tile_pool(name="io", bufs=2))
        one = ctx.enter_context(tc.tile_pool(name="one", bufs=1))
        idxp = ctx.enter_context(tc.tile_pool(name="idxp", bufs=6))
        gbp = ctx.enter_context(tc.tile_pool(name="gbp", bufs=6))
        gbi = ctx.enter_context(tc.tile_pool(name="gbi", bufs=3))
        psum = ctx.enter_context(tc.tile_pool(name="psum", bufs=2, space="PSUM"))
        psum1 = ctx.enter_context(tc.tile_pool(name="psum1", bufs=2, space="PSUM"))
        psum2 = ctx.enter_context(tc.tile_pool(name="psum2", bufs=2, space="PSUM"))

        nc.gpsimd.load_library(mlp)

        f0 = pers.tile([P, cfg.t, 64], F32, tag="f0")
        f1 = pers.tile([P, cfg.t, 64], F32, tag="f1")
        f2 = pers.tile([P, cfg.t, 64], F32, tag="f2")
        tbl = pers.tile([P, cfg.t + 1, 64], F32, tag="tbl")
        dinv_s = pers.tile([P, cfg.t], F32, tag="dinv")
        Ws = {w: pers.tile([cfg.in_feats if w in ("W1", "W4") else H, H],
                           F32, tag=w, name=w + "_s")
              for w in ("W1", "W2", "W3", "W4")}
        bs = {b: pers.tile([H, 1], F32, tag=b, name=b + "_s")
              for b in ("b1", "b2", "b3", "b4")}
        ident = pers.tile([P, P], F32, tag="ident")
        sid3 = pers.tile([P, P], F32, tag="sid3")
        sid075 = pers.tile([P, P], F32, tag="sid075")
        sidm15 = pers.tile([P, P], F32, tag="sidm15")

        tb_ins = [dram.tile([cfg.tp, 64], F32, name=f"tb_in{r}")
                  for r in range(2)]
        tb_fulls = [dram.tile([cfg.tp * cfg.nc, 64], F32, addr_space="Shared",
                              name=f"tb_full{r}") for r in range(2)]
        agg = pers.tile([P, cfg.t, 64], F32, tag="agg")
        iota_f = pers.tile([P, P], F32, tag="iota_f")

        for w in Ws:
            nc.sync.dma_start(Ws[w][:], W_d[w][:])
        for b in bs:
            nc.sync.dma_start(bs[b][:], b_d[b][:])
        nc.sync.dma_start(dinv_s[:], dinv_d[:])
        make_identity(nc, ident[:])
        nc.vector.tensor_scalar_mul(sid3[:], ident[:], 3.0)
        nc.vector.tensor_scalar_mul(sid075[:], ident[:], 0.75)
        nc.vector.tensor_scalar_mul(sidm15[:], ident[:], -1.5)
        nc.gpsimd.memset(tbl[:, cfg.t, :], 0.0)
        ioti = pers.tile([P, P], mybir.dt.int32, tag="ioti")
        nc.gpsimd.iota(ioti[:], pattern=[[1, P]], base=0, channel_multiplier=0)
        nc.vector.tensor_copy(iota_f[:], ioti[:])

        # ---- phase 1: MLP -> f0 node-major
        CH = cfg.mm_chunk
        for j0 in range(0, cfg.sp, CH):
            w = min(CH, cfg.sp - j0)
            xc = io.tile([cfg.in_feats, CH], F32, tag="xc")
            nc.sync.dma_start(xc[:, :w], xT_d[:, j0:j0 + w])
            ps1 = psum.tile([H, CH], F32, tag="A")
            nc.tensor.matmul(ps1[:, :w], Ws["W1"][:], xc[:, :w],
                             start=True, stop=True)
            h1c = io.tile([H, CH], F32, tag="h1c")
            nc.scalar.activation(h1c[:, :w], ps1[:, :w], relu, bias=bs["b1"][:])
            ps2 = psum.tile([H, CH], F32, tag="B")
            nc.tensor.matmul(ps2[:, :w], Ws["W2"][:], h1c[:, :w],
                             start=True, stop=True)
            h2c = io.tile([H, CH], F32, tag="h2c")
            nc.scalar.activation(h2c[:, :w], ps2[:, :w], relu, bias=bs["b2"][:])
            for i in range(w // P):
                t = (j0 + i * P) // P
                ps3 = psum1.tile([P, 64], F32, tag="C")
                nc.tensor.transpose(ps3[:], h2c[:, i * P:(i + 1) * P],
                                    ident[:H, :H])
                nc.scalar.activation(f0[:, t, :], ps3[:], cp)

        # ---- message passing rounds
        for rnd, (fprev, fnext) in enumerate([(f0, f1), (f1, f2)]):
            tb_in, tb_full = tb_ins[rnd], tb_fulls[rnd]
            nc.vector.tensor_tensor(
                tbl[:, :cfg.t, :], fprev[:],
                dinv_s[:, :, None].to_broadcast([P, cfg.t, 64]),
                mybir.AluOpType.mult)
            nc.sync.dma_start(
                tb_in[:].rearrange("(t p) f -> p t f", p=P), tbl[:])
            nc.gpsimd.collective_compute(
                "AllGather", mybir.AluOpType.bypass,
                replica_groups=[list(range(cfg.nc))],
                ins=[tb_in[:]], outs=[tb_full[:]])
            nc.gpsimd.memset(agg[:], 0.0)
            goff = 0
            for (c, btiles, spans) in plan:
                gi = idxp.tile([P, cfg.max_span_tiles * 8], I16, tag="gi")
                dwv = idxp.tile([P, cfg.max_span_tiles], F32, tag="dwv")
                nc.sync.dma_start(gi[:, :btiles * 8],
                                  gidx_d[:, goff * 8:(goff + btiles) * 8])
                nc.sync.dma_start(dwv[:, :btiles],
                                  dstw_d[:, goff:goff + btiles])
                gb = gbp.tile([P, cfg.max_span_tiles, 64], F32, tag="gb")
                ni = btiles * P
                nc.gpsimd.dma_gather(
                    gb[:, :btiles, :],
                    tb_full[c * cfg.chunk:(c + 1) * cfg.chunk, :],
                    gi[:, :btiles * 8], ni, ni, 64)
                ind = gbi.tile([P, cfg.max_span_tiles, P], F32, tag="ind")
                nc.vector.tensor_tensor(
                    ind[:, :btiles, :],
                    iota_f[:, None, :].to_broadcast([P, btiles, P]),
                    dwv[:, :btiles, None].to_broadcast([P, btiles, P]),
                    mybir.AluOpType.is_equal)
                for (toff, nt, ww) in spans:
                    pw = psum2.tile([P, 64], F32, tag="D")
                    for i in range(nt):
                        nc.tensor.matmul(pw[:], ind[:, toff + i, :],
                                         gb[:, toff + i, :],
                                         start=(i == 0), stop=(i == nt - 1))
                    nc.vector.tensor_tensor(agg[:, ww, :], agg[:, ww, :],
                                            pw[:], mybir.AluOpType.add)
                goff += btiles
            nc.vector.tensor_tensor(
                tbl[:, :cfg.t, :], agg[:],
                dinv_s[:, :, None].to_broadcast([P, cfg.t, 64]),
                mybir.AluOpType.mult)
            nc.vector.tensor_tensor(fnext[:], fprev[:], tbl[:, :cfg.t, :],
                                    mybir.AluOpType.subtract)

        # ---- filters + output MLPs
        nc.vector.tensor_tensor(f0[:], f0[:], f1[:], mybir.AluOpType.subtract)
        for j0 in range(0, cfg.sp, CH):
            w = min(CH, cfg.sp - j0)
            zl = psum.tile([H, CH], F32, tag="A")
            z1 = psum.tile([H, CH], F32, tag="B")
            z2 = psum1.tile([H, CH], F32, tag="C")
            for i in range(w // P):
                t = (j0 + i * P) // P
                cs = slice(i * P, (i + 1) * P)
                nc.tensor.matmul(zl[:, cs], f0[:, t, :], sid3[:],
                                 start=True, stop=False)
                nc.tensor.matmul(zl[:, cs], f2[:, t, :], sid075[:],
                                 start=False, stop=True)
                nc.tensor.matmul(z1[:, cs], f1[:, t, :], sid3[:],
                                 start=True, stop=False)
                nc.tensor.matmul(z1[:, cs], f2[:, t, :], sidm15[:],
                                 start=False, stop=True)
                nc.tensor.matmul(z2[:, cs], f2[:, t, :], sid075[:],
                                 start=True, stop=True)
            zlc = io.tile([H, CH], F32, tag="zlc")
            zhc = io.tile([P, CH], F32, tag="zhc")
            nc.scalar.activation(zlc[:, :w], zl[:, :w], cp)
            nc.scalar.activation(zhc[:H, :w], z1[:, :w], cp)
            nc.scalar.activation(zhc[H:, :w], z2[:, :w], cp)
            pl = psum1.tile([H, CH], F32, tag="C")
            ph = psum.tile([H, CH], F32, tag="A")
            nc.tensor.matmul(pl[:, :w], Ws["W3"][:], zlc[:, :w],
                             start=True, stop=True)
            nc.tensor.matmul(ph[:, :w], Ws["W4"][:], zhc[:, :w],
                             start=True, stop=True)
            ol = io.tile([H, CH], F32, tag="ol")
            oh = io.tile([H, CH], F32, tag="oh")
            nc.scalar.activation(ol[:, :w], pl[:, :w], relu, bias=bs["b3"][:])
            nc.scalar.activation(oh[:, :w], ph[:, :w], relu, bias=bs["b4"][:])
            nc.sync.dma_start(outl_d[:, j0:j0 + w], ol[:, :w])
            nc.sync.dma_start(outh_d[:, j0:j0 + w], oh[:, :w])

    nc.compile()
    return nc


# ---------------------------------------------------------------- driver

_CACHE = {}


def run(cfg, inputs, run_fn=None, **spmd_kwargs):
    in_maps, plan, total_tiles = preprocess(cfg, **inputs)
    key = (cfg.n_nodes, cfg.n_edges, total_tiles,
           tuple((c, b, tuple(s)) for c, b, s in plan))
    if key not in _CACHE:
        _CACHE[key] = build_nc(cfg, plan, total_tiles)
    nc = _CACHE[key]
    if run_fn is not None:
        results = run_fn(nc, in_maps)
        res = None
    else:
        res = run_bass_kernel_spmd(nc, in_maps, core_ids=list(range(cfg.nc)), **spmd_kwargs)
        results = res.results
    h_l = np.zeros((cfg.n_nodes, cfg.h), np.float32)
    h_h = np.zeros((cfg.n_nodes, cfg.h), np.float32)
    for c in range(cfg.nc):
        lo = c * cfg.shard
        h_l[lo:lo + cfg.shard] = results[c]["out_l"].T[:cfg.shard]
        h_h[lo:lo + cfg.shard] = results[c]["out_h"].T[:cfg.shard]
    return h_l, h_h, res


def kernel(in_feat, src, dst, W1, b1, W2, b2, W3, b3, W4, b4):
    cfg = Cfg(100000, 1600000, 128, 64, 8)
    h_l, h_h, _ = run(cfg, dict(
        in_feat=np.asarray(in_feat, np.float32),
        src=np.asarray(src, np.int64), dst=np.asarray(dst, np.int64),
        W1=np.asarray(W1, np.float32), b1=np.asarray(b1, np.float32),
        W2=np.asarray(W2, np.float32), b2=np.asarray(b2, np.float32),
        W3=np.asarray(W3, np.float32), b3=np.asarray(b3, np.float32),
        W4=np.asarray(W4, np.float32), b4=np.asarray(b4, np.float32)))
    return h_l, h_h
